# revision 27
# baseline (speedup 1.0000x reference)
"""GAT GNN (edge features) Trainium2 kernel — 8-core SPMD, v2.

Sharding: nodes by dst range (6250/core, padded 6400). Table rows are 512B
(256 bf16): per-layer features are kept in a rotated basis h@R_l whose last
column equals Wc_l@att_src_l, so the per-edge s_src logit is just column 255
of the gathered row (no extra embedded scalar -> 512B rows, 33% less gather
and AllGather traffic than 768B). All weight algebra (R_l^{-1} Wc_l R_{l+1}
folds, s_dst columns, We@att_edge projections) is folded on the host; the
per-edge attention bias A = edge_attr @ (We_l att_edge_l) is computed on the
host as well (it is layer-input independent).

Per layer: gather 512B rows (num_idxs_reg-trimmed) -> logits (A + G[:,255]
+ onehot-expanded s_dst via DVE mult+reduce) -> exp/leaky-relu -> Se ->
PSUM numerator+denominator (denominator as column 256 of the same PSUM
tile) -> normalize -> PE-transpose -> fused mm (R^-1 Wc R fold + s_dst
column) -> AllGather into parity ping-pong tables (overlaps next groups).
"""
import sys

sys.path.insert(0, "/opt/trn_rl_repo")

import numpy as np

NEG_SLOPE = 0.2
EPS = 1e-16
NC = 8
HID = 256
EDGE_DIM = 768
OUT_DIM = 256
N_LAYERS = 6


def make_cfg(n_nodes=50000, n_edges=400000):
    c = {}
    c["N"] = n_nodes
    c["E"] = n_edges
    c["D_CORE"] = n_nodes // NC
    c["D_PAD"] = -(-c["D_CORE"] // 128) * 128
    if (c["D_PAD"] // 128) % 2:
        c["D_PAD"] += 128          # even group count so HALF is 128-aligned
    c["HALF"] = c["D_PAD"] // 2
    c["TBL"] = NC * c["HALF"]
    assert c["TBL"] < 32768
    c["NG"] = c["D_PAD"] // 128
    # Each table half is AllGathered in two contiguous pieces so the
    # collective pipeline starts earlier and only the small tail piece
    # (groups G2..NG-1) is exposed at the layer boundary.  Row layout of
    # each half is piece-major (piece A's 8 cores, then piece B's).
    c["G1"] = 13                                      # T0 piece-A groups
    c["AR0"] = c["G1"] * 128
    c["BR0"] = c["HALF"] - c["AR0"]
    c["G2"] = (c["NG"] // 2) + 13
    c["AR"] = (c["G2"] - c["NG"] // 2) * 128          # T1 piece-A rows/core
    c["BR"] = c["D_PAD"] - c["HALF"] - c["AR"]        # T1 piece-B rows/core
    return c


# ---------------- host planner ----------------
def plan(cfg, edge_index, merge=1):
    """Slot space is t-major: slot = ((t*NG + g)*NBT + b)*128 + p.  Gather
    calls cover `merge` consecutive groups of one table half; only the last
    group's trailing pad is trimmed (middle pads gather row 0 harmlessly)."""
    src = np.asarray(edge_index[0], np.int64)
    dst = np.asarray(edge_index[1], np.int64)
    DC, HALF, NG = cfg["D_CORE"], cfg["HALF"], cfg["NG"]
    AR, BR = cfg["AR"], cfg["BR"]
    AR0, BR0 = cfg["AR0"], cfg["BR0"]
    assert NG % merge == 0

    per_core = []
    nbt = 1
    for c in range(NC):
        m = (dst >= c * DC) & (dst < (c + 1) * DC)
        eid = np.nonzero(m)[0]
        es, ed = src[eid], dst[eid] - c * DC
        et = ((es % DC) >= HALF).astype(np.int64)
        cs, ls = es // DC, es % DC
        # Both halves are piece-major (piece A's 8 cores, then piece B's),
        # core-major within each piece (matches the 2-piece AllGathers).
        erow_t0 = np.where(ls < AR0,
                           cs * AR0 + ls,
                           NC * AR0 + cs * BR0 + (ls - AR0))
        erow_t1 = np.where(ls < HALF + AR,
                           cs * AR + (ls - HALF),
                           NC * AR + cs * BR + (ls - HALF - AR))
        erow = np.where(et == 0, erow_t0, erow_t1)
        g = ed // 128
        per_core.append((eid, es, ed, et, erow, g))
        cnt = np.zeros((NG, 2), np.int64)
        np.add.at(cnt, (g, et), 1)
        nbt = max(nbt, int(-(-cnt.max() // 128)), 1)
    NBT = nbt
    NBINS = NG * 2 * NBT
    NSLOT = NBINS * 128
    NJ = NG // merge
    NCALLS = 2 * NJ

    gidx = np.full((NC, NSLOT), -1, np.int16)
    ngrp = np.zeros((NC, 2, NG), np.int32)        # real slots per (t, g)
    onehT = np.zeros((NC, 128, NSLOT), np.int8)   # [dst_local, slot]
    perm = np.full((NC, NSLOT), -1, np.int64)
    for c in range(NC):
        eid, es, ed, et, erow, g = per_core[c]
        for gg in range(NG):
            for t in (0, 1):
                sel = np.nonzero((g == gg) & (et == t))[0]
                base = ((t * NG + gg) * NBT) * 128
                ngrp[c, t, gg] = max(len(sel), 1)
                if len(sel) == 0:
                    continue
                slots = base + np.arange(len(sel))
                gidx[c, slots] = erow[sel].astype(np.int16)
                perm[c, slots] = eid[sel]
                onehT[c, ed[sel] - gg * 128, slots] = 1
    # merged-call trim counts: full middle groups + last group's real count.
    # Ucode contract: num_idxs_reg == count of idx >= 0, and only TRAILING
    # pads may be -1 -> pads below the trim point become row 0 (harmless).
    nreal = np.zeros((NC, NCALLS), np.int32)
    CLM = merge * NBT * 128
    for c in range(NC):
        for t in (0, 1):
            for j in range(NJ):
                nr = ((merge - 1) * NBT * 128
                      + ngrp[c, t, j * merge + merge - 1])
                nreal[c, t * NJ + j] = nr
                s = (t * NG + j * merge) * NBT * 128
                seg = gidx[c, s:s + nr]
                seg[seg < 0] = 0
    return dict(NBT=NBT, NBINS=NBINS, NSLOT=NSLOT, NCALLS=NCALLS,
                MERGE=merge, NJ=NJ, minr=nreal.min(axis=0),
                gidx=gidx, nreal=nreal, oneh=onehT, perm=perm)


def wrap_idx16(gidx, call_len):
    """[NSLOT] -> [128, NSLOT//16] with per-call 16-partition wrap."""
    ncalls = gidx.shape[0] // call_len
    blk = gidx.reshape(ncalls, call_len // 16, 16).transpose(2, 0, 1)
    flat = blk.reshape(16, ncalls * (call_len // 16))
    return np.tile(flat, (8, 1))


# ---------------- host weight folding ----------------
def fold_weights(inputs):
    """R_l rotations + fused per-layer rhs matrices, all in f64.

    Table basis: T^(i) = h^(i) @ R_i, with R_i[:, 255] = Wc_i @ att_src_i so
    s_src == gathered column 255.  R_i = H_i @ diag(1,..,1, beta*n) with H_i
    a Householder reflector, so R_i^{-1} is exact.
    """
    W1 = np.float64(inputs["W1"])
    W2 = np.float64(inputs["W2"])
    Wc = np.float64(inputs["Wc"])
    We = np.float64(inputs["We"])
    a_s = np.float64(inputs["att_src"])
    a_d = np.float64(inputs["att_dst"])
    a_e = np.float64(inputs["att_edge"])
    bias = np.float64(inputs["bias"])
    W3 = np.float64(inputs["W3"])

    R = []
    Rinv = []
    for i in range(N_LAYERS):
        v = Wc[i] @ a_s[i]
        n = np.linalg.norm(v)
        u = v / n
        beta = -1.0 if u[HID - 1] > 0 else 1.0
        w = u.copy()
        w[HID - 1] -= beta
        H = np.eye(HID) - 2.0 * np.outer(w, w) / (w @ w)
        # H @ e_last = beta*u  ->  R[:,255] = H[:,255] * (beta*n) = u*n = v
        Ri = H.copy()
        Ri[:, HID - 1] *= beta * n
        Rii = H.copy()                      # R^-1 = diag(1,..,1/(beta n)) @ H
        Rii[HID - 1, :] /= beta * n
        R.append(Ri)
        Rinv.append(Rii)

    v_d = [Wc[i] @ a_d[i] for i in range(N_LAYERS)]

    # mm matrices: index 0 = h0 producer (x @ W1W2 -> T^(0));
    # index 1+i = applied after layer i's aggregation.
    M = np.zeros((N_LAYERS + 1, HID, 258))
    brow = np.zeros((N_LAYERS + 1, 258))
    W12 = W1 @ W2
    M[0, :, 0:HID] = W12 @ R[0]
    M[0, :, HID] = W12 @ v_d[0]
    for i in range(N_LAYERS - 1):
        M[1 + i, :, 0:HID] = Rinv[i] @ Wc[i] @ R[i + 1]
        M[1 + i, :, HID] = Rinv[i] @ Wc[i] @ v_d[i + 1]
        brow[1 + i, 0:HID] = bias[i] @ R[i + 1]
        brow[1 + i, HID] = bias[i] @ v_d[i + 1]
    M[N_LAYERS, :, 0:HID] = Rinv[N_LAYERS - 1] @ Wc[N_LAYERS - 1]
    brow[N_LAYERS, 0:HID] = bias[N_LAYERS - 1]

    wal = np.einsum("lkh,lh->lk", We, a_e)          # [L, EDGE_DIM]
    W3p = W3[:HID] + W3[HID:]                        # [HID, OUT]
    return dict(R=R, Rinv=Rinv, M=M, brow=brow, wal=wal, W3p=W3p)


# ---------------- host-side input prep ----------------
def prep_inputs(cfg, pl, fw, inputs):
    x = np.asarray(inputs["x"], np.float32)
    ea = np.asarray(inputs["edge_attr"], np.float32)
    DC, DP = cfg["D_CORE"], cfg["D_PAD"]
    NSLOT, NBT, NBINS = pl["NSLOT"], pl["NBT"], pl["NBINS"]
    ml = __import__("ml_dtypes")
    bf16 = ml.bfloat16
    f8 = ml.float8_e4m3

    # per-edge attention bias, all layers at once: [E, L]
    A_full = ea @ np.float32(fw["wal"]).T

    M = np.float32(fw["M"])                          # [7, 256, 258]
    Mb = np.ascontiguousarray(
        M.reshape(N_LAYERS + 1, 2, 128, 258).transpose(2, 0, 1, 3)
    ).astype(bf16)                                   # [128, 7, 2, 258]
    brow = np.ascontiguousarray(
        np.broadcast_to(np.float32(fw["brow"])[None], (128, N_LAYERS + 1, 258))
    ).astype(bf16)
    W3p = np.ascontiguousarray(
        np.float32(fw["W3p"]).reshape(2, 128, OUT_DIM).transpose(1, 0, 2)
    ).astype(bf16)                                   # [128, 2, 256]

    common = dict(Mb=Mb, brow=brow, W3p=W3p)
    maps = []
    for c in range(NC):
        xs = np.zeros((DP, HID), np.float32)
        xs[:DC] = x[c * DC:(c + 1) * DC]
        m = dict(common)
        m["xT"] = np.ascontiguousarray(xs.T).astype(bf16)
        m["gidx"] = wrap_idx16(pl["gidx"][c], pl["MERGE"] * NBT * 128)
        m["nreal"] = pl["nreal"][c][None, :].astype(np.int32)
        # A in device layout [128, L, NSLOT//128]
        Ac = np.zeros((NSLOT, N_LAYERS), np.float32)
        real = pl["perm"][c] >= 0
        Ac[real] = A_full[pl["perm"][c][real]]
        m["A"] = np.ascontiguousarray(
            Ac.reshape(NSLOT // 128, 128, N_LAYERS).transpose(1, 2, 0)
        ).astype(bf16)
        # per-bin transposed onehot [slot_in_bin(p), dst_col], f8
        oh = pl["oneh"][c]
        oh_se = np.zeros((128, NSLOT), np.int8)
        for b in range(NBINS):
            oh_se[:, b * 128:(b + 1) * 128] = oh[:, b * 128:(b + 1) * 128].T
        m["oneh"] = oh_se.astype(f8)
        maps.append(m)
    return maps


# ---------------- numpy emulation (plan/fold validation) ----------------
def emulate(cfg, inputs, pl, fw):
    x = np.asarray(inputs["x"], np.float32)
    ea = np.asarray(inputs["edge_attr"], np.float32)
    DC, DP, HALF, TBL, NG = (cfg["D_CORE"], cfg["D_PAD"], cfg["HALF"],
                             cfg["TBL"], cfg["NG"])
    NSLOT, NBT = pl["NSLOT"], pl["NBT"]
    M = np.float32(fw["M"])
    brow = np.float32(fw["brow"])
    W3p = np.float32(fw["W3p"])

    A_full = ea @ np.float32(fw["wal"]).T
    A = np.zeros((NC, NSLOT, N_LAYERS), np.float32)
    for c in range(NC):
        real = pl["perm"][c] >= 0
        A[c][real] = A_full[pl["perm"][c][real]]

    # h0 phase
    mt = np.zeros((NC, DP, 257), np.float32)
    for c in range(NC):
        xs = np.zeros((DP, HID), np.float32)
        xs[:DC] = x[c * DC:(c + 1) * DC]
        mt[c] = xs @ M[0, :, 0:257] + brow[0, 0:257]

    slot_g = (np.arange(NSLOT) // (128 * NBT)) % NG
    out = np.zeros((NC, DP, OUT_DIM), np.float32)
    for i in range(N_LAYERS):
        # tables from mt
        agin = mt[:, :, 0:HID]
        sdst = mt[:, :, HID]
        AR, BR = cfg["AR"], cfg["BR"]
        AR0, BR0 = cfg["AR0"], cfg["BR0"]
        T0 = np.concatenate(
            [agin[:, :AR0].reshape(NC * AR0, HID),
             agin[:, AR0:HALF].reshape(NC * BR0, HID)], 0)
        T1 = np.concatenate(
            [agin[:, HALF:HALF + AR].reshape(NC * AR, HID),
             agin[:, HALF + AR:].reshape(NC * BR, HID)], 0)
        mt2 = np.zeros((NC, DP, 257), np.float32)
        for c in range(NC):
            gi = pl["gidx"][c].astype(np.int64)
            valid = pl["perm"][c] >= 0
            slot_t = np.arange(NSLOT) // (NG * NBT * 128)
            G = np.zeros((NSLOT, HID), np.float32)
            G[valid & (slot_t == 0)] = T0[gi[valid & (slot_t == 0)]]
            G[valid & (slot_t == 1)] = T1[gi[valid & (slot_t == 1)]]
            ssrc = G[:, HID - 1]
            oh = pl["oneh"][c].astype(np.float32)    # [dst_local, slot]
            sdsel = np.zeros(NSLOT, np.float32)
            for gg in range(NG):
                sl = slot_g == gg
                sdsel[sl] = oh[:, sl].T @ sdst[c, gg * 128:(gg + 1) * 128]
            alpha = ssrc + sdsel + A[c, :, i]
            eac = np.maximum(np.exp(alpha), np.exp(NEG_SLOPE * alpha))
            U = np.zeros((DP, HID), np.float32)
            dns = np.zeros(DP, np.float32)
            Se = oh * eac[None, :]
            for gg in range(NG):
                sl = slot_g == gg
                U[gg * 128:(gg + 1) * 128] = Se[:, sl] @ G[sl]
                dns[gg * 128:(gg + 1) * 128] = Se[:, sl].sum(1)
            U = U / (dns + EPS)[:, None]
            if i < N_LAYERS - 1:
                mt2[c] = U @ M[1 + i, :, 0:257] + brow[1 + i, 0:257]
                mt2[c, DC:] = 0.0
            else:
                h7 = U @ M[1 + i, :, 0:HID] + brow[1 + i, 0:HID]
                out[c] = np.maximum(h7, 0.0) @ W3p
        mt = mt2
    return np.concatenate([out[c, :DC] for c in range(NC)], 0)


# ---------------- device kernel ----------------
def build(cfg, pl, queues=4, debug_taps=False, zero_g=False, no_ag=False,
          gq=4):
    import concourse.bass as bass
    import concourse.tile as tile
    import concourse.mybir as mybir
    from concourse import bacc
    from concourse.masks import make_identity

    f32, bf16, i16, i32 = (mybir.dt.float32, mybir.dt.bfloat16,
                           mybir.dt.int16, mybir.dt.int32)
    f8 = mybir.dt.float8e4
    ACT = mybir.ActivationFunctionType
    ALU = mybir.AluOpType

    DP, HALF, TBL, NG = cfg["D_PAD"], cfg["HALF"], cfg["TBL"], cfg["NG"]
    G2, AR = cfg["G2"], cfg["AR"]
    G1, AR0 = cfg["G1"], cfg["AR0"]
    NBT, NSLOT, NCALLS = pl["NBT"], pl["NSLOT"], pl["NCALLS"]
    CL = NBT * 128
    NKC = HID // 128
    NJ = NSLOT // 128

    nc = bacc.Bacc(None, target_bir_lowering=False, debug=False,
                   num_swdge_queues=queues)

    # inputs
    xT = nc.dram_tensor("xT", [HID, DP], bf16, kind="ExternalInput")
    gidxD = nc.dram_tensor("gidx", [128, NSLOT // 16], i16, kind="ExternalInput")
    nrealD = nc.dram_tensor("nreal", [1, NCALLS], i32, kind="ExternalInput")
    onehD = nc.dram_tensor("oneh", [128, NSLOT], f8, kind="ExternalInput")
    AD = nc.dram_tensor("A", [128, N_LAYERS, NJ], bf16, kind="ExternalInput")
    MbD = nc.dram_tensor("Mb", [128, N_LAYERS + 1, NKC, 258], bf16,
                         kind="ExternalInput")
    browD = nc.dram_tensor("brow", [128, N_LAYERS + 1, 258], bf16,
                           kind="ExternalInput")
    W3pD = nc.dram_tensor("W3p", [128, NKC, OUT_DIM], bf16,
                          kind="ExternalInput")
    outD = nc.dram_tensor("out", [DP, OUT_DIM], f32, kind="ExternalOutput")
    dbg = {}
    if debug_taps:
        for nm, shp, dt in [("dbg_T0", [TBL, HID], bf16),
                            ("dbg_srep", [128, 128], bf16),
                            ("dbg_sc", [128, 16 * NBT], f32),
                            ("dbg_G", [128, NBT * HID], bf16),
                            ("dbg_Se", [128, 2 * NBT * 128], bf16),
                            ("dbg_gps", [128, 258], f32),
                            ("dbg_hn", [128, HID], bf16),
                            ("dbg_mt", [128, 258], f32)]:
            dbg[nm] = nc.dram_tensor(nm, shp, dt, kind="ExternalOutput")

    # internals (ping-pong tables/agin by layer parity)
    aginD = [nc.dram_tensor(f"agin{p}", [DP, HID], bf16) for p in (0, 1)]
    T0D = [nc.dram_tensor(f"T0_{p}", [TBL, HID], bf16, addr_space="Shared")
           for p in (0, 1)]
    T1D = [nc.dram_tensor(f"T1_{p}", [TBL, HID], bf16, addr_space="Shared")
           for p in (0, 1)]
    sdTD = [nc.dram_tensor(f"sdT{p}", [NG, 128], bf16) for p in (0, 1)]

    rg = [list(range(NC))]

    with tile.TileContext(nc) as tc:
        with (
            tc.tile_pool(name="res", bufs=1) as res,
            tc.tile_pool(name="lw", bufs=4) as lw,
            tc.tile_pool(name="gp", bufs=10) as gp,
            tc.tile_pool(name="sep", bufs=6) as sep,
            tc.tile_pool(name="exm", bufs=4) as exmp,
            tc.tile_pool(name="sc", bufs=6) as scp,
            tc.tile_pool(name="hn", bufs=5) as hnp,
            tc.tile_pool(name="hT", bufs=5) as hTp,
            tc.tile_pool(name="hex", bufs=5) as hex_,
            tc.tile_pool(name="acc", bufs=4, space="PSUM") as accp,
            tc.tile_pool(name="dns", bufs=2, space="PSUM") as dnsp,
            tc.tile_pool(name="tpp", bufs=2, space="PSUM") as tpp,
        ):
            # resident inputs
            gidx_sb = res.tile([128, NSLOT // 16], i16)
            nc.sync.dma_start(gidx_sb[:], gidxD[:])
            nreal_sb = res.tile([1, NCALLS], i32)
            nc.sync.dma_start(nreal_sb[:], nrealD[:])
            oneh_sb = res.tile([128, NSLOT], f8)
            nc.sync.dma_start(oneh_sb[:], onehD[:])
            A_sb = res.tile([128, N_LAYERS, NJ], bf16)
            nc.sync.dma_start(A_sb[:], AD[:])
            Mb_sb = res.tile([128, N_LAYERS + 1, NKC, 258], bf16)
            nc.sync.dma_start(Mb_sb[:], MbD[:])
            brow_sb = res.tile([128, N_LAYERS + 1, 258], bf16)
            nc.sync.dma_start(brow_sb[:], browD[:])
            W3p_sb = res.tile([128, NKC, OUT_DIM], bf16)
            nc.sync.dma_start(W3p_sb[:], W3pD[:])

            ident = res.tile([128, 128], bf16)
            make_identity(nc, ident[:])
            ones_col = res.tile([128, 1], bf16)
            nc.vector.memset(ones_col[:], 1.0)
            sdst_bf = res.tile([128, NG], bf16)
            nreal_reg = nc.gpsimd.alloc_register("nreal_reg")

            def zero_pad_suffix(G, call, nbins):
                """Sim-only: zero pad slots (logical tiles are NaN there).
                On HW the pool priming below keeps stale pads finite, which
                is all the masked (oneh=0) reads need."""
                if not zero_g:
                    return
                b0 = int(pl["minr"][call]) // 128
                if b0 < nbins:
                    nc.vector.memset(
                        G[:, b0:nbins, :].rearrange("p a b -> p (a b)"), 0.0)
            if not zero_g:
                for _ in range(10):
                    gt = gp.tile([128, pl["MERGE"] * NBT, HID], bf16, tag="G",
                                 name="gprime")
                    nc.vector.memset(gt[:].rearrange("p a b -> p (a b)"), 0.0)

            def mm_retire(src_sb, li, g, wr_parity):
                """matmul src^T @ M[li] (+brow) -> table row + sdst col."""
                mt = accp.tile([128, 258], f32, tag="acc", name="mt")
                for kc in range(NKC):
                    nc.tensor.matmul(mt[:, 0:257], src_sb[:, kc, :],
                                     Mb_sb[:, li, kc, 0:257],
                                     start=(kc == 0), stop=(kc == NKC - 1))
                hx = hex_.tile([128, HID], bf16, tag="hx")
                nc.vector.tensor_tensor(out=hx[:], in0=mt[:, 0:HID],
                                        in1=brow_sb[:, li, 0:HID], op=ALU.add)
                nc.vector.tensor_tensor(out=sdst_bf[:, g:g + 1],
                                        in0=mt[:, HID:HID + 1],
                                        in1=brow_sb[:, li, HID:HID + 1],
                                        op=ALU.add)
                nc.sync.dma_start(aginD[wr_parity][g * 128:(g + 1) * 128, :],
                                  hx[:])

            def emit_ags(g, wr_parity):
                if no_ag:
                    return
                if g == G1 - 1:
                    nc.gpsimd.collective_compute(
                        "AllGather", ALU.bypass, replica_groups=rg,
                        ins=[aginD[wr_parity][0:AR0, :]],
                        outs=[T0D[wr_parity][0:NC * AR0, :]])
                if g == NG // 2 - 1:
                    nc.gpsimd.collective_compute(
                        "AllGather", ALU.bypass, replica_groups=rg,
                        ins=[aginD[wr_parity][AR0:HALF, :]],
                        outs=[T0D[wr_parity][NC * AR0:TBL, :]])
                if g == G2 - 1:
                    nc.gpsimd.collective_compute(
                        "AllGather", ALU.bypass, replica_groups=rg,
                        ins=[aginD[wr_parity][HALF:HALF + AR, :]],
                        outs=[T1D[wr_parity][0:NC * AR, :]])
                if g == NG - 1:
                    nc.gpsimd.collective_compute(
                        "AllGather", ALU.bypass, replica_groups=rg,
                        ins=[aginD[wr_parity][HALF + AR:DP, :]],
                        outs=[T1D[wr_parity][NC * AR:TBL, :]])

            def sdst_transpose(parity):
                sdT_ps = tpp.tile([128, 128], bf16, tag="tp", name="sdT_ps")
                nc.tensor.transpose(sdT_ps[0:NG, :], sdst_bf[:], ident[:])
                sdT = hTp.tile([128, 128], bf16, tag="sdT", name="sdT")
                nc.vector.tensor_copy(sdT[0:NG, :], sdT_ps[0:NG, :])
                nc.sync.dma_start(sdTD[parity][:], sdT[0:NG, :])

            # ---------- h0: T^(0) = x @ W1W2R0 ----------
            with nc.named_scope("h0"):
                for g in range(NG):
                    xt = lw.tile([128, NKC, 128], bf16, tag="xt")
                    for kc in range(NKC):
                        nc.sync.dma_start(
                            xt[:, kc, :],
                            xT[kc * 128:(kc + 1) * 128, g * 128:(g + 1) * 128])
                    mm_retire(xt, 0, g, 0)
                    emit_ags(g, 0)
                sdst_transpose(0)

            # ---------- layers ----------
            MERGE, NJ = pl["MERGE"], pl["NJ"]
            CLM = MERGE * CL
            for i in range(N_LAYERS):
                last = i == N_LAYERS - 1
                rd, wr = i % 2, (i + 1) % 2
                if debug_taps and i == 0:
                    nc.sync.dma_start(dbg["dbg_T0"][:], T0D[0][:])
                with nc.named_scope(f"eg{i}"):
                    for j in range(NJ):
                        Gm = [None, None]
                        for t in (0, 1):
                            call = t * NJ + j
                            G = gp.tile([128, MERGE * NBT, HID], bf16,
                                        tag="G")
                            zero_pad_suffix(G, call, MERGE * NBT)
                            nc.gpsimd.reg_load(nreal_reg,
                                               nreal_sb[0:1, call:call + 1])
                            nc.gpsimd.dma_gather(
                                out_ap=G[:],
                                in_ap=(T0D[rd][:] if t == 0 else T1D[rd][:]),
                                idxs_ap=gidx_sb[:, call * (CLM // 16):
                                                (call + 1) * (CLM // 16)],
                                num_idxs=CLM, num_idxs_reg=nreal_reg,
                                elem_size=HID,
                                queue_num=(t * 2 + (j % 2)) % gq)
                            Gm[t] = G
                        for gsub in range(MERGE):
                            g = j * MERGE + gsub
                            dbg_this = debug_taps and i == 0 and g == 0
                            srep = lw.tile([128, 128], bf16, tag="srep",
                                           name="srep")
                            nc.sync.dma_start(
                                srep[:],
                                sdTD[rd][g:g + 1, :].to_broadcast((128, 128)))
                            if dbg_this:
                                nc.sync.dma_start(dbg["dbg_srep"][:], srep[:])
                            gps = accp.tile([128, 258], f32, tag="acc",
                                            name="gps")
                            dns = dnsp.tile([128, 1], f32, tag="dns",
                                            name="dns")
                            for t in (0, 1):
                                bb = (t * NG + g) * NBT
                                Gv = Gm[t][:, gsub * NBT:(gsub + 1) * NBT, :]
                                oh_v = oneh_sb[:, bb * 128:(bb + NBT) * 128] \
                                    .rearrange("p (a b) -> p a b", b=128)
                                exm = exmp.tile([128, NBT, 128], bf16,
                                                tag="exm")
                                nc.vector.tensor_tensor(
                                    out=exm[:], in0=oh_v,
                                    in1=srep[:, None, :]
                                    .to_broadcast([128, NBT, 128]),
                                    op=ALU.mult)
                                ex = scp.tile([128, NBT], f32, tag=f"ex{t}")
                                nc.vector.tensor_reduce(
                                    ex[:], exm[:], axis=mybir.AxisListType.X,
                                    op=ALU.add)
                                beta = scp.tile([128, NBT], f32,
                                                tag=f"beta{t}")
                                nc.vector.tensor_tensor(
                                    out=beta[:],
                                    in0=A_sb[:, i, bb:bb + NBT],
                                    in1=Gv[:, :, HID - 1], op=ALU.add)
                                alpha = scp.tile([128, NBT], f32,
                                                 tag=f"alpha{t}")
                                nc.vector.tensor_tensor(
                                    out=alpha[:], in0=beta[:], in1=ex[:],
                                    op=ALU.add)
                                e1 = scp.tile([128, NBT], f32, tag=f"e1{t}")
                                nc.scalar.activation(e1[:], alpha[:], ACT.Exp)
                                e2 = scp.tile([128, NBT], f32, tag=f"e2{t}")
                                nc.scalar.activation(e2[:], alpha[:], ACT.Exp,
                                                     scale=NEG_SLOPE)
                                eac = scp.tile([128, NBT], f32, tag=f"eac{t}")
                                nc.vector.tensor_tensor(
                                    out=eac[:], in0=e1[:], in1=e2[:],
                                    op=ALU.max)
                                Se = sep.tile([128, NBT, 128], bf16,
                                              tag=f"Se{t}")
                                for b in range(NBT):
                                    nc.scalar.activation(
                                        Se[:, b, :], oh_v[:, b, :],
                                        ACT.Copy, scale=eac[:, b:b + 1])
                                if dbg_this:
                                    sc = dbg["dbg_sc"]
                                    nc.sync.dma_start(
                                        sc[:, (0 + t) * NBT:(1 + t) * NBT],
                                        ex[:])
                                    nc.sync.dma_start(
                                        sc[:, (2 + t) * NBT:(3 + t) * NBT],
                                        beta[:])
                                    nc.sync.dma_start(
                                        sc[:, (4 + t) * NBT:(5 + t) * NBT],
                                        alpha[:])
                                    nc.sync.dma_start(
                                        sc[:, (6 + t) * NBT:(7 + t) * NBT],
                                        eac[:])
                                    nc.sync.dma_start(
                                        dbg["dbg_Se"][:, t * NBT * 128:
                                                      (t + 1) * NBT * 128],
                                        Se[:].rearrange("p a b -> p (a b)"))
                                    if t == 0:
                                        nc.sync.dma_start(
                                            dbg["dbg_G"][:],
                                            Gv[:].rearrange(
                                                "p a b -> p (a b)"))
                                for b in range(NBT):
                                    ii = t * NBT + b
                                    nc.tensor.matmul(
                                        gps[:, 0:HID], Se[:, b, :],
                                        Gv[:, b, :],
                                        start=(ii == 0),
                                        stop=(ii == 2 * NBT - 1))
                                    nc.tensor.matmul(
                                        dns[:], Se[:, b, :], ones_col[:],
                                        start=(ii == 0),
                                        stop=(ii == 2 * NBT - 1))
                            dcol = scp.tile([128, 1], f32, tag="dcol")
                            nc.vector.tensor_scalar_add(dcol[:], dns[:], EPS)
                            rcol = scp.tile([128, 1], f32, tag="rcol")
                            nc.vector.reciprocal(rcol[:], dcol[:])
                            hn = hnp.tile([128, HID], bf16, tag="hn")
                            if dbg_this:
                                gcp = hex_.tile([128, 258], f32, tag="gcp",
                                                name="gcp")
                                nc.vector.tensor_copy(gcp[:, 0:HID],
                                                      gps[:, 0:HID])
                                nc.vector.tensor_copy(gcp[:, HID:HID + 1],
                                                      dns[:])
                                nc.sync.dma_start(dbg["dbg_gps"][:], gcp[:])
                            nc.scalar.activation(hn[:], gps[:, 0:HID],
                                                 ACT.Copy, scale=rcol[:, 0:1])
                            if dbg_this:
                                nc.sync.dma_start(dbg["dbg_hn"][:], hn[:])
                            tp = tpp.tile([128, NKC, 128], bf16, tag="tp",
                                          name="tp")
                            for kc in range(NKC):
                                nc.tensor.transpose(
                                    tp[:, kc, :],
                                    hn[:, kc * 128:(kc + 1) * 128], ident[:])
                            hT = hTp.tile([128, NKC, 128], bf16, tag="hT")
                            nc.vector.tensor_copy(hT[:], tp[:])
                            if not last:
                                mm_retire(hT, 1 + i, g, wr)
                                emit_ags(g, wr)
                            else:
                                mt = accp.tile([128, 258], f32, tag="acc",
                                               name="mt6")
                                for kc in range(NKC):
                                    nc.tensor.matmul(
                                        mt[:, 0:HID], hT[:, kc, :],
                                        Mb_sb[:, 1 + i, kc, 0:HID],
                                        start=(kc == 0), stop=(kc == NKC - 1))
                                h6x = hex_.tile([128, HID], f32, tag="h6x",
                                                name="h6x")
                                nc.vector.tensor_tensor(
                                    out=h6x[:], in0=mt[:, 0:HID],
                                    in1=brow_sb[:, 1 + i, 0:HID], op=ALU.add)
                                rh = hnp.tile([128, HID], bf16, tag="rh",
                                              name="rh")
                                nc.scalar.activation(rh[:], h6x[:], ACT.Relu)
                                tp2 = tpp.tile([128, NKC, 128], bf16,
                                               tag="tp", name="tp2")
                                for kc in range(NKC):
                                    nc.tensor.transpose(
                                        tp2[:, kc, :],
                                        rh[:, kc * 128:(kc + 1) * 128],
                                        ident[:])
                                rhT = hTp.tile([128, NKC, 128], bf16,
                                               tag="hT", name="rhT")
                                nc.vector.tensor_copy(rhT[:], tp2[:])
                                ops = accp.tile([128, 258], f32, tag="acc",
                                                name="ops")
                                for kc in range(NKC):
                                    nc.tensor.matmul(
                                        ops[:, 0:OUT_DIM], rhT[:, kc, :],
                                        W3p_sb[:, kc, :],
                                        start=(kc == 0), stop=(kc == NKC - 1))
                                outf = hex_.tile([128, OUT_DIM], f32,
                                                 tag="outf", name="outf")
                                nc.vector.tensor_copy(outf[:],
                                                      ops[:, 0:OUT_DIM])
                                nc.sync.dma_start(
                                    outD[g * 128:(g + 1) * 128, :], outf[:])
                    if not last:
                        sdst_transpose(wr)

    nc.compile()
    return nc


_CACHE = {}


def kernel(**inputs) -> np.ndarray:
    from concourse.bass_utils import run_bass_kernel_spmd

    cfg = make_cfg()
    ei = np.asarray(inputs["edge_index"])
    pl = plan(cfg, ei)
    key = ("nc", pl["NBT"])
    if key not in _CACHE:
        _CACHE[key] = build(cfg, pl)
    nc = _CACHE[key]
    fw = fold_weights(inputs)
    maps = prep_inputs(cfg, pl, fw, inputs)
    res = run_bass_kernel_spmd(nc, maps, core_ids=list(range(NC)))
    DC = cfg["D_CORE"]
    return np.concatenate([res.results[c]["out"][:DC] for c in range(NC)],
                          0).astype(np.float32)



# revision 29
# speedup vs baseline: 1.0209x; 1.0209x over previous
"""GAT GNN (edge features) Trainium2 kernel — 8-core SPMD, v2.

Sharding: nodes by dst range (6250/core, padded 6400). Table rows are 512B
(256 bf16): per-layer features are kept in a rotated basis h@R_l whose last
column equals Wc_l@att_src_l, so the per-edge s_src logit is just column 255
of the gathered row (no extra embedded scalar -> 512B rows, 33% less gather
and AllGather traffic than 768B). All weight algebra (R_l^{-1} Wc_l R_{l+1}
folds, s_dst columns, We@att_edge projections) is folded on the host; the
per-edge attention bias A = edge_attr @ (We_l att_edge_l) is computed on the
host as well (it is layer-input independent).

Per layer: gather 512B rows (num_idxs_reg-trimmed) -> logits (A + G[:,255]
+ onehot-expanded s_dst via DVE mult+reduce) -> exp/leaky-relu -> Se ->
PSUM numerator+denominator (denominator as column 256 of the same PSUM
tile) -> normalize -> PE-transpose -> fused mm (R^-1 Wc R fold + s_dst
column) -> AllGather into parity ping-pong tables (overlaps next groups).
"""
import sys

sys.path.insert(0, "/opt/trn_rl_repo")

import numpy as np

NEG_SLOPE = 0.2
EPS = 1e-16
NC = 8
HID = 256
EDGE_DIM = 768
OUT_DIM = 256
N_LAYERS = 6


def make_cfg(n_nodes=50000, n_edges=400000):
    c = {}
    c["N"] = n_nodes
    c["E"] = n_edges
    c["D_CORE"] = n_nodes // NC
    c["D_PAD"] = -(-c["D_CORE"] // 128) * 128
    if (c["D_PAD"] // 128) % 2:
        c["D_PAD"] += 128          # even group count so HALF is 128-aligned
    c["HALF"] = c["D_PAD"] // 2
    c["TBL"] = NC * c["HALF"]
    assert c["TBL"] < 32768
    c["NG"] = c["D_PAD"] // 128
    # Each table half is AllGathered in two contiguous pieces so the
    # collective pipeline starts earlier and only the small tail piece
    # (groups G2..NG-1) is exposed at the layer boundary.  Row layout of
    # each half is piece-major (piece A's 8 cores, then piece B's).
    c["G1"] = 13                                      # T0 piece-A groups
    c["AR0"] = c["G1"] * 128
    c["BR0"] = c["HALF"] - c["AR0"]
    c["G2"] = (c["NG"] // 2) + 13
    c["AR"] = (c["G2"] - c["NG"] // 2) * 128          # T1 piece-A rows/core
    c["BR"] = c["D_PAD"] - c["HALF"] - c["AR"]        # T1 piece-B rows/core
    return c


# ---------------- host planner ----------------
def plan(cfg, edge_index, merge=1):
    """Slot space is t-major: slot = ((t*NG + g)*NBT + b)*128 + p.  Gather
    calls cover `merge` consecutive groups of one table half; only the last
    group's trailing pad is trimmed (middle pads gather row 0 harmlessly)."""
    src = np.asarray(edge_index[0], np.int64)
    dst = np.asarray(edge_index[1], np.int64)
    DC, HALF, NG = cfg["D_CORE"], cfg["HALF"], cfg["NG"]
    AR, BR = cfg["AR"], cfg["BR"]
    AR0, BR0 = cfg["AR0"], cfg["BR0"]
    assert NG % merge == 0

    per_core = []
    nbt = 1
    for c in range(NC):
        m = (dst >= c * DC) & (dst < (c + 1) * DC)
        eid = np.nonzero(m)[0]
        es, ed = src[eid], dst[eid] - c * DC
        et = ((es % DC) >= HALF).astype(np.int64)
        cs, ls = es // DC, es % DC
        # Both halves are piece-major (piece A's 8 cores, then piece B's),
        # core-major within each piece (matches the 2-piece AllGathers).
        erow_t0 = np.where(ls < AR0,
                           cs * AR0 + ls,
                           NC * AR0 + cs * BR0 + (ls - AR0))
        erow_t1 = np.where(ls < HALF + AR,
                           cs * AR + (ls - HALF),
                           NC * AR + cs * BR + (ls - HALF - AR))
        erow = np.where(et == 0, erow_t0, erow_t1)
        g = ed // 128
        per_core.append((eid, es, ed, et, erow, g))
        cnt = np.zeros((NG, 2), np.int64)
        np.add.at(cnt, (g, et), 1)
        nbt = max(nbt, int(-(-cnt.max() // 128)), 1)
    NBT = nbt
    NBINS = NG * 2 * NBT
    NSLOT = NBINS * 128
    NJ = NG // merge
    NCALLS = 2 * NJ

    gidx = np.full((NC, NSLOT), -1, np.int16)
    ngrp = np.zeros((NC, 2, NG), np.int32)        # real slots per (t, g)
    onehT = np.zeros((NC, 128, NSLOT), np.int8)   # [dst_local, slot]
    perm = np.full((NC, NSLOT), -1, np.int64)
    for c in range(NC):
        eid, es, ed, et, erow, g = per_core[c]
        for gg in range(NG):
            for t in (0, 1):
                sel = np.nonzero((g == gg) & (et == t))[0]
                base = ((t * NG + gg) * NBT) * 128
                ngrp[c, t, gg] = max(len(sel), 1)
                if len(sel) == 0:
                    continue
                slots = base + np.arange(len(sel))
                gidx[c, slots] = erow[sel].astype(np.int16)
                perm[c, slots] = eid[sel]
                onehT[c, ed[sel] - gg * 128, slots] = 1
    # merged-call trim counts: full middle groups + last group's real count.
    # Ucode contract: num_idxs_reg == count of idx >= 0, and only TRAILING
    # pads may be -1 -> pads below the trim point become row 0 (harmless).
    nreal = np.zeros((NC, NCALLS), np.int32)
    CLM = merge * NBT * 128
    for c in range(NC):
        for t in (0, 1):
            for j in range(NJ):
                nr = ((merge - 1) * NBT * 128
                      + ngrp[c, t, j * merge + merge - 1])
                nreal[c, t * NJ + j] = nr
                s = (t * NG + j * merge) * NBT * 128
                seg = gidx[c, s:s + nr]
                seg[seg < 0] = 0
    return dict(NBT=NBT, NBINS=NBINS, NSLOT=NSLOT, NCALLS=NCALLS,
                MERGE=merge, NJ=NJ, minr=nreal.min(axis=0),
                gidx=gidx, nreal=nreal, oneh=onehT, perm=perm)


def wrap_idx16(gidx, call_len):
    """[NSLOT] -> [128, NSLOT//16] with per-call 16-partition wrap."""
    ncalls = gidx.shape[0] // call_len
    blk = gidx.reshape(ncalls, call_len // 16, 16).transpose(2, 0, 1)
    flat = blk.reshape(16, ncalls * (call_len // 16))
    return np.tile(flat, (8, 1))


# ---------------- host weight folding ----------------
def fold_weights(inputs):
    """R_l rotations + fused per-layer rhs matrices, all in f64.

    Table basis: T^(i) = h^(i) @ R_i, with R_i[:, 255] = Wc_i @ att_src_i so
    s_src == gathered column 255.  R_i = H_i @ diag(1,..,1, beta*n) with H_i
    a Householder reflector, so R_i^{-1} is exact.
    """
    W1 = np.float64(inputs["W1"])
    W2 = np.float64(inputs["W2"])
    Wc = np.float64(inputs["Wc"])
    We = np.float64(inputs["We"])
    a_s = np.float64(inputs["att_src"])
    a_d = np.float64(inputs["att_dst"])
    a_e = np.float64(inputs["att_edge"])
    bias = np.float64(inputs["bias"])
    W3 = np.float64(inputs["W3"])

    R = []
    Rinv = []
    for i in range(N_LAYERS):
        v = Wc[i] @ a_s[i]
        n = np.linalg.norm(v)
        u = v / n
        beta = -1.0 if u[HID - 1] > 0 else 1.0
        w = u.copy()
        w[HID - 1] -= beta
        H = np.eye(HID) - 2.0 * np.outer(w, w) / (w @ w)
        # H @ e_last = beta*u  ->  R[:,255] = H[:,255] * (beta*n) = u*n = v
        Ri = H.copy()
        Ri[:, HID - 1] *= beta * n
        Rii = H.copy()                      # R^-1 = diag(1,..,1/(beta n)) @ H
        Rii[HID - 1, :] /= beta * n
        R.append(Ri)
        Rinv.append(Rii)

    v_d = [Wc[i] @ a_d[i] for i in range(N_LAYERS)]

    # mm matrices: index 0 = h0 producer (x @ W1W2 -> T^(0));
    # index 1+i = applied after layer i's aggregation.
    M = np.zeros((N_LAYERS + 1, HID, 258))
    brow = np.zeros((N_LAYERS + 1, 258))
    W12 = W1 @ W2
    M[0, :, 0:HID] = W12 @ R[0]
    M[0, :, HID] = W12 @ v_d[0]
    for i in range(N_LAYERS - 1):
        M[1 + i, :, 0:HID] = Rinv[i] @ Wc[i] @ R[i + 1]
        M[1 + i, :, HID] = Rinv[i] @ Wc[i] @ v_d[i + 1]
        brow[1 + i, 0:HID] = bias[i] @ R[i + 1]
        brow[1 + i, HID] = bias[i] @ v_d[i + 1]
    M[N_LAYERS, :, 0:HID] = Rinv[N_LAYERS - 1] @ Wc[N_LAYERS - 1]
    brow[N_LAYERS, 0:HID] = bias[N_LAYERS - 1]

    wal = np.einsum("lkh,lh->lk", We, a_e)          # [L, EDGE_DIM]
    W3p = W3[:HID] + W3[HID:]                        # [HID, OUT]
    return dict(R=R, Rinv=Rinv, M=M, brow=brow, wal=wal, W3p=W3p)


# ---------------- host-side input prep ----------------
def prep_inputs(cfg, pl, fw, inputs):
    x = np.asarray(inputs["x"], np.float32)
    ea = np.asarray(inputs["edge_attr"], np.float32)
    DC, DP = cfg["D_CORE"], cfg["D_PAD"]
    NSLOT, NBT, NBINS = pl["NSLOT"], pl["NBT"], pl["NBINS"]
    ml = __import__("ml_dtypes")
    bf16 = ml.bfloat16
    f8 = ml.float8_e4m3

    # per-edge attention bias, all layers at once: [E, L]
    A_full = ea @ np.float32(fw["wal"]).T

    M = np.float32(fw["M"])                          # [7, 256, 258]
    Mb = np.ascontiguousarray(
        M.reshape(N_LAYERS + 1, 2, 128, 258).transpose(2, 0, 1, 3)
    ).astype(bf16)                                   # [128, 7, 2, 258]
    brow = np.ascontiguousarray(
        np.broadcast_to(np.float32(fw["brow"])[None], (128, N_LAYERS + 1, 258))
    ).astype(bf16)
    W3p = np.ascontiguousarray(
        np.float32(fw["W3p"]).reshape(2, 128, OUT_DIM).transpose(1, 0, 2)
    ).astype(bf16)                                   # [128, 2, 256]

    common = dict(Mb=Mb, brow=brow, W3p=W3p)
    maps = []
    for c in range(NC):
        xs = np.zeros((DP, HID), np.float32)
        xs[:DC] = x[c * DC:(c + 1) * DC]
        m = dict(common)
        m["xT"] = np.ascontiguousarray(xs.T).astype(bf16)
        m["gidx"] = wrap_idx16(pl["gidx"][c], pl["MERGE"] * NBT * 128)
        m["nreal"] = pl["nreal"][c][None, :].astype(np.int32)
        # A in device layout [128, L, NSLOT//128]
        Ac = np.zeros((NSLOT, N_LAYERS), np.float32)
        real = pl["perm"][c] >= 0
        Ac[real] = A_full[pl["perm"][c][real]]
        m["A"] = np.ascontiguousarray(
            Ac.reshape(NSLOT // 128, 128, N_LAYERS).transpose(1, 2, 0)
        ).astype(bf16)
        # per-bin transposed onehot [slot_in_bin(p), dst_col], f8
        oh = pl["oneh"][c]
        oh_se = np.zeros((128, NSLOT), np.int8)
        for b in range(NBINS):
            oh_se[:, b * 128:(b + 1) * 128] = oh[:, b * 128:(b + 1) * 128].T
        m["oneh"] = oh_se.astype(f8)
        maps.append(m)
    return maps


# ---------------- numpy emulation (plan/fold validation) ----------------
def emulate(cfg, inputs, pl, fw):
    x = np.asarray(inputs["x"], np.float32)
    ea = np.asarray(inputs["edge_attr"], np.float32)
    DC, DP, HALF, TBL, NG = (cfg["D_CORE"], cfg["D_PAD"], cfg["HALF"],
                             cfg["TBL"], cfg["NG"])
    NSLOT, NBT = pl["NSLOT"], pl["NBT"]
    M = np.float32(fw["M"])
    brow = np.float32(fw["brow"])
    W3p = np.float32(fw["W3p"])

    A_full = ea @ np.float32(fw["wal"]).T
    A = np.zeros((NC, NSLOT, N_LAYERS), np.float32)
    for c in range(NC):
        real = pl["perm"][c] >= 0
        A[c][real] = A_full[pl["perm"][c][real]]

    # h0 phase
    mt = np.zeros((NC, DP, 257), np.float32)
    for c in range(NC):
        xs = np.zeros((DP, HID), np.float32)
        xs[:DC] = x[c * DC:(c + 1) * DC]
        mt[c] = xs @ M[0, :, 0:257] + brow[0, 0:257]

    slot_g = (np.arange(NSLOT) // (128 * NBT)) % NG
    out = np.zeros((NC, DP, OUT_DIM), np.float32)
    for i in range(N_LAYERS):
        # tables from mt
        agin = mt[:, :, 0:HID]
        sdst = mt[:, :, HID]
        AR, BR = cfg["AR"], cfg["BR"]
        AR0, BR0 = cfg["AR0"], cfg["BR0"]
        T0 = np.concatenate(
            [agin[:, :AR0].reshape(NC * AR0, HID),
             agin[:, AR0:HALF].reshape(NC * BR0, HID)], 0)
        T1 = np.concatenate(
            [agin[:, HALF:HALF + AR].reshape(NC * AR, HID),
             agin[:, HALF + AR:].reshape(NC * BR, HID)], 0)
        mt2 = np.zeros((NC, DP, 257), np.float32)
        for c in range(NC):
            gi = pl["gidx"][c].astype(np.int64)
            valid = pl["perm"][c] >= 0
            slot_t = np.arange(NSLOT) // (NG * NBT * 128)
            G = np.zeros((NSLOT, HID), np.float32)
            G[valid & (slot_t == 0)] = T0[gi[valid & (slot_t == 0)]]
            G[valid & (slot_t == 1)] = T1[gi[valid & (slot_t == 1)]]
            ssrc = G[:, HID - 1]
            oh = pl["oneh"][c].astype(np.float32)    # [dst_local, slot]
            sdsel = np.zeros(NSLOT, np.float32)
            for gg in range(NG):
                sl = slot_g == gg
                sdsel[sl] = oh[:, sl].T @ sdst[c, gg * 128:(gg + 1) * 128]
            alpha = ssrc + sdsel + A[c, :, i]
            eac = np.maximum(np.exp(alpha), np.exp(NEG_SLOPE * alpha))
            U = np.zeros((DP, HID), np.float32)
            dns = np.zeros(DP, np.float32)
            Se = oh * eac[None, :]
            for gg in range(NG):
                sl = slot_g == gg
                U[gg * 128:(gg + 1) * 128] = Se[:, sl] @ G[sl]
                dns[gg * 128:(gg + 1) * 128] = Se[:, sl].sum(1)
            U = U / (dns + EPS)[:, None]
            if i < N_LAYERS - 1:
                mt2[c] = U @ M[1 + i, :, 0:257] + brow[1 + i, 0:257]
                mt2[c, DC:] = 0.0
            else:
                h7 = U @ M[1 + i, :, 0:HID] + brow[1 + i, 0:HID]
                out[c] = np.maximum(h7, 0.0) @ W3p
        mt = mt2
    return np.concatenate([out[c, :DC] for c in range(NC)], 0)


# ---------------- device kernel ----------------
def build(cfg, pl, queues=4, debug_taps=False, zero_g=False, no_ag=False,
          gq=4):
    import concourse.bass as bass
    import concourse.tile as tile
    import concourse.mybir as mybir
    from concourse import bacc
    from concourse.masks import make_identity

    f32, bf16, i16, i32 = (mybir.dt.float32, mybir.dt.bfloat16,
                           mybir.dt.int16, mybir.dt.int32)
    f8 = mybir.dt.float8e4
    ACT = mybir.ActivationFunctionType
    ALU = mybir.AluOpType

    DP, HALF, TBL, NG = cfg["D_PAD"], cfg["HALF"], cfg["TBL"], cfg["NG"]
    G2, AR = cfg["G2"], cfg["AR"]
    G1, AR0 = cfg["G1"], cfg["AR0"]
    NBT, NSLOT, NCALLS = pl["NBT"], pl["NSLOT"], pl["NCALLS"]
    CL = NBT * 128
    NKC = HID // 128
    NJ = NSLOT // 128

    nc = bacc.Bacc(None, target_bir_lowering=False, debug=False,
                   num_swdge_queues=queues)

    # inputs
    xT = nc.dram_tensor("xT", [HID, DP], bf16, kind="ExternalInput")
    gidxD = nc.dram_tensor("gidx", [128, NSLOT // 16], i16, kind="ExternalInput")
    nrealD = nc.dram_tensor("nreal", [1, NCALLS], i32, kind="ExternalInput")
    onehD = nc.dram_tensor("oneh", [128, NSLOT], f8, kind="ExternalInput")
    AD = nc.dram_tensor("A", [128, N_LAYERS, NJ], bf16, kind="ExternalInput")
    MbD = nc.dram_tensor("Mb", [128, N_LAYERS + 1, NKC, 258], bf16,
                         kind="ExternalInput")
    browD = nc.dram_tensor("brow", [128, N_LAYERS + 1, 258], bf16,
                           kind="ExternalInput")
    W3pD = nc.dram_tensor("W3p", [128, NKC, OUT_DIM], bf16,
                          kind="ExternalInput")
    outD = nc.dram_tensor("out", [DP, OUT_DIM], f32, kind="ExternalOutput")
    dbg = {}
    if debug_taps:
        for nm, shp, dt in [("dbg_T0", [TBL, HID], bf16),
                            ("dbg_srep", [128, 128], bf16),
                            ("dbg_sc", [128, 16 * NBT], f32),
                            ("dbg_G", [128, NBT * HID], bf16),
                            ("dbg_Se", [128, 2 * NBT * 128], bf16),
                            ("dbg_gps", [128, 258], f32),
                            ("dbg_hn", [128, HID], bf16),
                            ("dbg_mt", [128, 258], f32)]:
            dbg[nm] = nc.dram_tensor(nm, shp, dt, kind="ExternalOutput")

    # internals (ping-pong tables/agin by layer parity)
    aginD = [nc.dram_tensor(f"agin{p}", [DP, HID], bf16) for p in (0, 1)]
    T0D = [nc.dram_tensor(f"T0_{p}", [TBL, HID], bf16, addr_space="Shared")
           for p in (0, 1)]
    T1D = [nc.dram_tensor(f"T1_{p}", [TBL, HID], bf16, addr_space="Shared")
           for p in (0, 1)]
    sdTD = [nc.dram_tensor(f"sdT{p}", [NG, 128], bf16) for p in (0, 1)]

    rg = [list(range(NC))]

    with tile.TileContext(nc) as tc:
        with (
            tc.tile_pool(name="res", bufs=1) as res,
            tc.tile_pool(name="lw", bufs=4) as lw,
            tc.tile_pool(name="gp", bufs=10) as gp,
            tc.tile_pool(name="sep", bufs=6) as sep,
            tc.tile_pool(name="exm", bufs=4) as exmp,
            tc.tile_pool(name="sc", bufs=6) as scp,
            tc.tile_pool(name="hn", bufs=5) as hnp,
            tc.tile_pool(name="hT", bufs=5) as hTp,
            tc.tile_pool(name="hex", bufs=5) as hex_,
            tc.tile_pool(name="acc", bufs=4, space="PSUM") as accp,
            tc.tile_pool(name="dns", bufs=2, space="PSUM") as dnsp,
            tc.tile_pool(name="tpp", bufs=2, space="PSUM") as tpp,
        ):
            # resident inputs
            gidx_sb = res.tile([128, NSLOT // 16], i16)
            nc.sync.dma_start(gidx_sb[:], gidxD[:])
            nreal_sb = res.tile([1, NCALLS], i32)
            nc.sync.dma_start(nreal_sb[:], nrealD[:])
            oneh_sb = res.tile([128, NSLOT], f8)
            nc.sync.dma_start(oneh_sb[:], onehD[:])
            A_sb = res.tile([128, N_LAYERS, NJ], bf16)
            nc.sync.dma_start(A_sb[:], AD[:])
            Mb_sb = res.tile([128, N_LAYERS + 1, NKC, 258], bf16)
            nc.sync.dma_start(Mb_sb[:], MbD[:])
            brow_sb = res.tile([128, N_LAYERS + 1, 258], bf16)
            nc.sync.dma_start(brow_sb[:], browD[:])
            W3p_sb = res.tile([128, NKC, OUT_DIM], bf16)
            nc.sync.dma_start(W3p_sb[:], W3pD[:])

            ident = res.tile([128, 128], bf16)
            make_identity(nc, ident[:])
            ones_col = res.tile([128, 1], bf16)
            nc.vector.memset(ones_col[:], 1.0)
            sdst_bf = res.tile([128, NG], bf16)
            nreal_reg = nc.gpsimd.alloc_register("nreal_reg")

            def zero_pad_suffix(G, call, nbins):
                """Sim-only: zero pad slots (logical tiles are NaN there).
                On HW the pool priming below keeps stale pads finite, which
                is all the masked (oneh=0) reads need."""
                if not zero_g:
                    return
                b0 = int(pl["minr"][call]) // 128
                if b0 < nbins:
                    nc.vector.memset(
                        G[:, b0:nbins, :].rearrange("p a b -> p (a b)"), 0.0)
            if not zero_g:
                for _ in range(10):
                    gt = gp.tile([128, pl["MERGE"] * NBT, HID], bf16, tag="G",
                                 name="gprime")
                    nc.vector.memset(gt[:].rearrange("p a b -> p (a b)"), 0.0)

            def mm_retire(src_sb, li, g, wr_parity):
                """matmul src^T @ M[li] (+brow) -> table row + sdst col."""
                mt = accp.tile([128, 258], f32, tag="acc", name="mt")
                for kc in range(NKC):
                    nc.tensor.matmul(mt[:, 0:257], src_sb[:, kc, :],
                                     Mb_sb[:, li, kc, 0:257],
                                     start=(kc == 0), stop=(kc == NKC - 1))
                hx = hex_.tile([128, HID], bf16, tag="hx")
                nc.vector.tensor_tensor(out=hx[:], in0=mt[:, 0:HID],
                                        in1=brow_sb[:, li, 0:HID], op=ALU.add)
                nc.vector.tensor_tensor(out=sdst_bf[:, g:g + 1],
                                        in0=mt[:, HID:HID + 1],
                                        in1=brow_sb[:, li, HID:HID + 1],
                                        op=ALU.add)
                nc.sync.dma_start(aginD[wr_parity][g * 128:(g + 1) * 128, :],
                                  hx[:])

            def emit_ags(g, wr_parity):
                if no_ag:
                    return
                if g == G1 - 1:
                    nc.gpsimd.collective_compute(
                        "AllGather", ALU.bypass, replica_groups=rg,
                        ins=[aginD[wr_parity][0:AR0, :]],
                        outs=[T0D[wr_parity][0:NC * AR0, :]])
                if g == NG // 2 - 1:
                    nc.gpsimd.collective_compute(
                        "AllGather", ALU.bypass, replica_groups=rg,
                        ins=[aginD[wr_parity][AR0:HALF, :]],
                        outs=[T0D[wr_parity][NC * AR0:TBL, :]])
                if g == G2 - 1:
                    nc.gpsimd.collective_compute(
                        "AllGather", ALU.bypass, replica_groups=rg,
                        ins=[aginD[wr_parity][HALF:HALF + AR, :]],
                        outs=[T1D[wr_parity][0:NC * AR, :]])
                if g == NG - 1:
                    nc.gpsimd.collective_compute(
                        "AllGather", ALU.bypass, replica_groups=rg,
                        ins=[aginD[wr_parity][HALF + AR:DP, :]],
                        outs=[T1D[wr_parity][NC * AR:TBL, :]])

            def sdst_transpose(parity):
                sdT_ps = tpp.tile([128, 128], bf16, tag="tp", name="sdT_ps")
                nc.tensor.transpose(sdT_ps[0:NG, :], sdst_bf[:], ident[:])
                sdT = hTp.tile([128, 128], bf16, tag="sdT", name="sdT")
                nc.vector.tensor_copy(sdT[0:NG, :], sdT_ps[0:NG, :])
                nc.sync.dma_start(sdTD[parity][:], sdT[0:NG, :])

            # ---------- h0: T^(0) = x @ W1W2R0 ----------
            with nc.named_scope("h0"):
                for g in range(NG):
                    xt = lw.tile([128, NKC, 128], bf16, tag="xt")
                    for kc in range(NKC):
                        nc.sync.dma_start(
                            xt[:, kc, :],
                            xT[kc * 128:(kc + 1) * 128, g * 128:(g + 1) * 128])
                    mm_retire(xt, 0, g, 0)
                    emit_ags(g, 0)
                sdst_transpose(0)

            # ---------- layers ----------
            MERGE, NJ = pl["MERGE"], pl["NJ"]
            CLM = MERGE * CL
            for i in range(N_LAYERS):
                last = i == N_LAYERS - 1
                rd, wr = i % 2, (i + 1) % 2
                if debug_taps and i == 0:
                    nc.sync.dma_start(dbg["dbg_T0"][:], T0D[0][:])
                with nc.named_scope(f"eg{i}"):
                    for j in range(NJ):
                        Gm = [None, None]
                        for t in (0, 1):
                            call = t * NJ + j
                            G = gp.tile([128, MERGE * NBT, HID], bf16,
                                        tag="G")
                            zero_pad_suffix(G, call, MERGE * NBT)
                            nc.gpsimd.reg_load(nreal_reg,
                                               nreal_sb[0:1, call:call + 1])
                            nc.gpsimd.dma_gather(
                                out_ap=G[:],
                                in_ap=(T0D[rd][:] if t == 0 else T1D[rd][:]),
                                idxs_ap=gidx_sb[:, call * (CLM // 16):
                                                (call + 1) * (CLM // 16)],
                                num_idxs=CLM, num_idxs_reg=nreal_reg,
                                elem_size=HID,
                                queue_num=(t * 2 + (j % 2)) % gq)
                            Gm[t] = G
                        for gsub in range(MERGE):
                            g = j * MERGE + gsub
                            dbg_this = debug_taps and i == 0 and g == 0
                            srep = lw.tile([128, 128], bf16, tag="srep",
                                           name="srep")
                            nc.sync.dma_start(
                                srep[:],
                                sdTD[rd][g:g + 1, :].to_broadcast((128, 128)))
                            if dbg_this:
                                nc.sync.dma_start(dbg["dbg_srep"][:], srep[:])
                            gps = accp.tile([128, 258], f32, tag="acc",
                                            name="gps")
                            dns = dnsp.tile([128, 1], f32, tag="dns",
                                            name="dns")
                            for t in (0, 1):
                                bb = (t * NG + g) * NBT
                                Gv = Gm[t][:, gsub * NBT:(gsub + 1) * NBT, :]
                                oh_v = oneh_sb[:, bb * 128:(bb + NBT) * 128] \
                                    .rearrange("p (a b) -> p a b", b=128)
                                exm = exmp.tile([128, NBT, 128], bf16,
                                                tag="exm")
                                nc.vector.tensor_tensor(
                                    out=exm[:], in0=oh_v,
                                    in1=srep[:, None, :]
                                    .to_broadcast([128, NBT, 128]),
                                    op=ALU.mult)
                                ex = scp.tile([128, NBT], f32, tag=f"ex{t}")
                                nc.vector.tensor_reduce(
                                    ex[:], exm[:], axis=mybir.AxisListType.X,
                                    op=ALU.add)
                                beta = scp.tile([128, NBT], f32,
                                                tag=f"beta{t}")
                                nc.vector.tensor_tensor(
                                    out=beta[:],
                                    in0=A_sb[:, i, bb:bb + NBT],
                                    in1=Gv[:, :, HID - 1], op=ALU.add)
                                alpha = scp.tile([128, NBT], f32,
                                                 tag=f"alpha{t}")
                                nc.vector.tensor_tensor(
                                    out=alpha[:], in0=beta[:], in1=ex[:],
                                    op=ALU.add)
                                e1 = scp.tile([128, NBT], f32, tag=f"e1{t}")
                                nc.scalar.activation(e1[:], alpha[:], ACT.Exp)
                                e2 = scp.tile([128, NBT], f32, tag=f"e2{t}")
                                nc.scalar.activation(e2[:], alpha[:], ACT.Exp,
                                                     scale=NEG_SLOPE)
                                eac = scp.tile([128, NBT], f32, tag=f"eac{t}")
                                nc.vector.tensor_tensor(
                                    out=eac[:], in0=e1[:], in1=e2[:],
                                    op=ALU.max)
                                Se = sep.tile([128, NBT, 128], bf16,
                                              tag=f"Se{t}")
                                for b in range(NBT):
                                    nc.scalar.activation(
                                        Se[:, b, :], oh_v[:, b, :],
                                        ACT.Copy, scale=eac[:, b:b + 1])
                                if dbg_this:
                                    sc = dbg["dbg_sc"]
                                    nc.sync.dma_start(
                                        sc[:, (0 + t) * NBT:(1 + t) * NBT],
                                        ex[:])
                                    nc.sync.dma_start(
                                        sc[:, (2 + t) * NBT:(3 + t) * NBT],
                                        beta[:])
                                    nc.sync.dma_start(
                                        sc[:, (4 + t) * NBT:(5 + t) * NBT],
                                        alpha[:])
                                    nc.sync.dma_start(
                                        sc[:, (6 + t) * NBT:(7 + t) * NBT],
                                        eac[:])
                                    nc.sync.dma_start(
                                        dbg["dbg_Se"][:, t * NBT * 128:
                                                      (t + 1) * NBT * 128],
                                        Se[:].rearrange("p a b -> p (a b)"))
                                    if t == 0:
                                        nc.sync.dma_start(
                                            dbg["dbg_G"][:],
                                            Gv[:].rearrange(
                                                "p a b -> p (a b)"))
                                for b in range(NBT):
                                    ii = t * NBT + b
                                    nc.tensor.matmul(
                                        gps[:, 0:HID], Se[:, b, :],
                                        Gv[:, b, :],
                                        start=(ii == 0),
                                        stop=(ii == 2 * NBT - 1))
                                    nc.tensor.matmul(
                                        dns[:], Se[:, b, :], ones_col[:],
                                        start=(ii == 0),
                                        stop=(ii == 2 * NBT - 1))
                            dcol = scp.tile([128, 1], f32, tag="dcol")
                            nc.vector.tensor_scalar_add(dcol[:], dns[:], EPS)
                            rcol = scp.tile([128, 1], f32, tag="rcol")
                            nc.vector.reciprocal(rcol[:], dcol[:])
                            hn = hnp.tile([128, HID], bf16, tag="hn")
                            if dbg_this:
                                gcp = hex_.tile([128, 258], f32, tag="gcp",
                                                name="gcp")
                                nc.vector.tensor_copy(gcp[:, 0:HID],
                                                      gps[:, 0:HID])
                                nc.vector.tensor_copy(gcp[:, HID:HID + 1],
                                                      dns[:])
                                nc.sync.dma_start(dbg["dbg_gps"][:], gcp[:])
                            nc.scalar.activation(hn[:], gps[:, 0:HID],
                                                 ACT.Copy, scale=rcol[:, 0:1])
                            if dbg_this:
                                nc.sync.dma_start(dbg["dbg_hn"][:], hn[:])
                            tp = tpp.tile([128, NKC, 128], bf16, tag="tp",
                                          name="tp")
                            for kc in range(NKC):
                                nc.tensor.transpose(
                                    tp[:, kc, :],
                                    hn[:, kc * 128:(kc + 1) * 128], ident[:])
                            hT = hTp.tile([128, NKC, 128], bf16, tag="hT")
                            nc.vector.tensor_copy(hT[:], tp[:])
                            if not last:
                                mm_retire(hT, 1 + i, g, wr)
                                emit_ags(g, wr)
                            else:
                                mt = accp.tile([128, 258], f32, tag="acc",
                                               name="mt6")
                                for kc in range(NKC):
                                    nc.tensor.matmul(
                                        mt[:, 0:HID], hT[:, kc, :],
                                        Mb_sb[:, 1 + i, kc, 0:HID],
                                        start=(kc == 0), stop=(kc == NKC - 1))
                                h6x = hex_.tile([128, HID], f32, tag="h6x",
                                                name="h6x")
                                nc.vector.tensor_tensor(
                                    out=h6x[:], in0=mt[:, 0:HID],
                                    in1=brow_sb[:, 1 + i, 0:HID], op=ALU.add)
                                rh = hnp.tile([128, HID], bf16, tag="rh",
                                              name="rh")
                                nc.scalar.activation(rh[:], h6x[:], ACT.Relu)
                                tp2 = tpp.tile([128, NKC, 128], bf16,
                                               tag="tp", name="tp2")
                                for kc in range(NKC):
                                    nc.tensor.transpose(
                                        tp2[:, kc, :],
                                        rh[:, kc * 128:(kc + 1) * 128],
                                        ident[:])
                                rhT = hTp.tile([128, NKC, 128], bf16,
                                               tag="hT", name="rhT")
                                nc.vector.tensor_copy(rhT[:], tp2[:])
                                ops = accp.tile([128, 258], f32, tag="acc",
                                                name="ops")
                                for kc in range(NKC):
                                    nc.tensor.matmul(
                                        ops[:, 0:OUT_DIM], rhT[:, kc, :],
                                        W3p_sb[:, kc, :],
                                        start=(kc == 0), stop=(kc == NKC - 1))
                                outf = hex_.tile([128, OUT_DIM], f32,
                                                 tag="outf", name="outf")
                                nc.vector.tensor_copy(outf[:],
                                                      ops[:, 0:OUT_DIM])
                                nc.sync.dma_start(
                                    outD[g * 128:(g + 1) * 128, :], outf[:])
                    if not last:
                        sdst_transpose(wr)

    nc.compile()
    return nc


_CACHE = {}


def kernel(**inputs) -> np.ndarray:
    from concourse.bass_utils import run_bass_kernel_spmd

    cfg = make_cfg()
    ei = np.asarray(inputs["edge_index"])
    pl = plan(cfg, ei)
    key = ("nc", pl["NBT"])
    if key not in _CACHE:
        _CACHE[key] = build(cfg, pl)
    nc = _CACHE[key]
    fw = fold_weights(inputs)
    maps = prep_inputs(cfg, pl, fw, inputs)
    res = run_bass_kernel_spmd(nc, maps, core_ids=list(range(NC)))
    DC = cfg["D_CORE"]
    return np.concatenate([res.results[c]["out"][:DC] for c in range(NC)],
                          0).astype(np.float32)



# revision 31
# speedup vs baseline: 1.0245x; 1.0035x over previous
"""GAT GNN (edge features) Trainium2 kernel — 8-core SPMD, v2.

Sharding: nodes by dst range (6250/core, padded 6400). Table rows are 512B
(256 bf16): per-layer features are kept in a rotated basis h@R_l whose last
column equals Wc_l@att_src_l, so the per-edge s_src logit is just column 255
of the gathered row (no extra embedded scalar -> 512B rows, 33% less gather
and AllGather traffic than 768B). All weight algebra (R_l^{-1} Wc_l R_{l+1}
folds, s_dst columns, We@att_edge projections) is folded on the host; the
per-edge attention bias A = edge_attr @ (We_l att_edge_l) is computed on the
host as well (it is layer-input independent).

Per layer: gather 512B rows (num_idxs_reg-trimmed) -> logits (A + G[:,255]
+ onehot-expanded s_dst via DVE mult+reduce) -> exp/leaky-relu -> Se ->
PSUM numerator+denominator (denominator as column 256 of the same PSUM
tile) -> normalize -> PE-transpose -> fused mm (R^-1 Wc R fold + s_dst
column) -> AllGather into parity ping-pong tables (overlaps next groups).
"""
import sys

sys.path.insert(0, "/opt/trn_rl_repo")

import numpy as np

NEG_SLOPE = 0.2
EPS = 1e-16
NC = 8
HID = 256
EDGE_DIM = 768
OUT_DIM = 256
N_LAYERS = 6


def make_cfg(n_nodes=50000, n_edges=400000):
    c = {}
    c["N"] = n_nodes
    c["E"] = n_edges
    c["D_CORE"] = n_nodes // NC
    c["D_PAD"] = -(-c["D_CORE"] // 128) * 128
    if (c["D_PAD"] // 128) % 2:
        c["D_PAD"] += 128          # even group count so HALF is 128-aligned
    c["HALF"] = c["D_PAD"] // 2
    c["TBL"] = NC * c["HALF"]
    assert c["TBL"] < 32768
    c["NG"] = c["D_PAD"] // 128
    # Each table half is AllGathered in two contiguous pieces so the
    # collective pipeline starts earlier and only the small tail piece
    # (groups G2..NG-1) is exposed at the layer boundary.  Row layout of
    # each half is piece-major (piece A's 8 cores, then piece B's).
    c["G1"] = 13                                      # T0 piece-A groups
    c["AR0"] = c["G1"] * 128
    c["BR0"] = c["HALF"] - c["AR0"]
    c["G2"] = (c["NG"] // 2) + 13
    c["AR"] = (c["G2"] - c["NG"] // 2) * 128          # T1 piece-A rows/core
    c["BR"] = c["D_PAD"] - c["HALF"] - c["AR"]        # T1 piece-B rows/core
    return c


# ---------------- host planner ----------------
def plan(cfg, edge_index, merge=1):
    """Slot space is t-major: slot = ((t*NG + g)*NBT + b)*128 + p.  Gather
    calls cover `merge` consecutive groups of one table half; only the last
    group's trailing pad is trimmed (middle pads gather row 0 harmlessly)."""
    src = np.asarray(edge_index[0], np.int64)
    dst = np.asarray(edge_index[1], np.int64)
    DC, HALF, NG = cfg["D_CORE"], cfg["HALF"], cfg["NG"]
    AR, BR = cfg["AR"], cfg["BR"]
    AR0, BR0 = cfg["AR0"], cfg["BR0"]
    assert NG % merge == 0

    per_core = []
    nbt = 1
    for c in range(NC):
        m = (dst >= c * DC) & (dst < (c + 1) * DC)
        eid = np.nonzero(m)[0]
        es, ed = src[eid], dst[eid] - c * DC
        et = ((es % DC) >= HALF).astype(np.int64)
        cs, ls = es // DC, es % DC
        # Both halves are piece-major (piece A's 8 cores, then piece B's),
        # core-major within each piece (matches the 2-piece AllGathers).
        erow_t0 = np.where(ls < AR0,
                           cs * AR0 + ls,
                           NC * AR0 + cs * BR0 + (ls - AR0))
        erow_t1 = np.where(ls < HALF + AR,
                           cs * AR + (ls - HALF),
                           NC * AR + cs * BR + (ls - HALF - AR))
        erow = np.where(et == 0, erow_t0, erow_t1)
        g = ed // 128
        per_core.append((eid, es, ed, et, erow, g))
        cnt = np.zeros((NG, 2), np.int64)
        np.add.at(cnt, (g, et), 1)
        nbt = max(nbt, int(-(-cnt.max() // 128)), 1)
    NBT = nbt
    NBINS = NG * 2 * NBT
    NSLOT = NBINS * 128
    NJ = NG // merge
    NCALLS = 2 * NJ

    gidx = np.full((NC, NSLOT), -1, np.int16)
    ngrp = np.zeros((NC, 2, NG), np.int32)        # real slots per (t, g)
    onehT = np.zeros((NC, 128, NSLOT), np.int8)   # [dst_local, slot]
    perm = np.full((NC, NSLOT), -1, np.int64)
    for c in range(NC):
        eid, es, ed, et, erow, g = per_core[c]
        for gg in range(NG):
            for t in (0, 1):
                sel = np.nonzero((g == gg) & (et == t))[0]
                base = ((t * NG + gg) * NBT) * 128
                ngrp[c, t, gg] = max(len(sel), 1)
                if len(sel) == 0:
                    continue
                slots = base + np.arange(len(sel))
                gidx[c, slots] = erow[sel].astype(np.int16)
                perm[c, slots] = eid[sel]
                onehT[c, ed[sel] - gg * 128, slots] = 1
    # merged-call trim counts: full middle groups + last group's real count.
    # Ucode contract: num_idxs_reg == count of idx >= 0, and only TRAILING
    # pads may be -1 -> pads below the trim point become row 0 (harmless).
    nreal = np.zeros((NC, NCALLS), np.int32)
    CLM = merge * NBT * 128
    for c in range(NC):
        for t in (0, 1):
            for j in range(NJ):
                nr = ((merge - 1) * NBT * 128
                      + ngrp[c, t, j * merge + merge - 1])
                nreal[c, t * NJ + j] = nr
                s = (t * NG + j * merge) * NBT * 128
                seg = gidx[c, s:s + nr]
                seg[seg < 0] = 0
    return dict(NBT=NBT, NBINS=NBINS, NSLOT=NSLOT, NCALLS=NCALLS,
                MERGE=merge, NJ=NJ, minr=nreal.min(axis=0),
                gidx=gidx, nreal=nreal, oneh=onehT, perm=perm)


def wrap_idx16(gidx, call_len):
    """[NSLOT] -> [128, NSLOT//16] with per-call 16-partition wrap."""
    ncalls = gidx.shape[0] // call_len
    blk = gidx.reshape(ncalls, call_len // 16, 16).transpose(2, 0, 1)
    flat = blk.reshape(16, ncalls * (call_len // 16))
    return np.tile(flat, (8, 1))


# ---------------- host weight folding ----------------
def fold_weights(inputs):
    """R_l rotations + fused per-layer rhs matrices, all in f64.

    Table basis: T^(i) = h^(i) @ R_i, with R_i[:, 255] = Wc_i @ att_src_i so
    s_src == gathered column 255.  R_i = H_i @ diag(1,..,1, beta*n) with H_i
    a Householder reflector, so R_i^{-1} is exact.
    """
    W1 = np.float64(inputs["W1"])
    W2 = np.float64(inputs["W2"])
    Wc = np.float64(inputs["Wc"])
    We = np.float64(inputs["We"])
    a_s = np.float64(inputs["att_src"])
    a_d = np.float64(inputs["att_dst"])
    a_e = np.float64(inputs["att_edge"])
    bias = np.float64(inputs["bias"])
    W3 = np.float64(inputs["W3"])

    R = []
    Rinv = []
    for i in range(N_LAYERS):
        v = Wc[i] @ a_s[i]
        n = np.linalg.norm(v)
        u = v / n
        beta = -1.0 if u[HID - 1] > 0 else 1.0
        w = u.copy()
        w[HID - 1] -= beta
        H = np.eye(HID) - 2.0 * np.outer(w, w) / (w @ w)
        # H @ e_last = beta*u  ->  R[:,255] = H[:,255] * (beta*n) = u*n = v
        Ri = H.copy()
        Ri[:, HID - 1] *= beta * n
        Rii = H.copy()                      # R^-1 = diag(1,..,1/(beta n)) @ H
        Rii[HID - 1, :] /= beta * n
        R.append(Ri)
        Rinv.append(Rii)

    v_d = [Wc[i] @ a_d[i] for i in range(N_LAYERS)]

    # mm matrices: index 0 = h0 producer (x @ W1W2 -> T^(0));
    # index 1+i = applied after layer i's aggregation.
    M = np.zeros((N_LAYERS + 1, HID, 258))
    brow = np.zeros((N_LAYERS + 1, 258))
    W12 = W1 @ W2
    M[0, :, 0:HID] = W12 @ R[0]
    M[0, :, HID] = W12 @ v_d[0]
    for i in range(N_LAYERS - 1):
        M[1 + i, :, 0:HID] = Rinv[i] @ Wc[i] @ R[i + 1]
        M[1 + i, :, HID] = Rinv[i] @ Wc[i] @ v_d[i + 1]
        brow[1 + i, 0:HID] = bias[i] @ R[i + 1]
        brow[1 + i, HID] = bias[i] @ v_d[i + 1]
    M[N_LAYERS, :, 0:HID] = Rinv[N_LAYERS - 1] @ Wc[N_LAYERS - 1]
    brow[N_LAYERS, 0:HID] = bias[N_LAYERS - 1]

    wal = np.einsum("lkh,lh->lk", We, a_e)          # [L, EDGE_DIM]
    W3p = W3[:HID] + W3[HID:]                        # [HID, OUT]
    return dict(R=R, Rinv=Rinv, M=M, brow=brow, wal=wal, W3p=W3p)


# ---------------- host-side input prep ----------------
def prep_inputs(cfg, pl, fw, inputs):
    x = np.asarray(inputs["x"], np.float32)
    ea = np.asarray(inputs["edge_attr"], np.float32)
    DC, DP = cfg["D_CORE"], cfg["D_PAD"]
    NSLOT, NBT, NBINS = pl["NSLOT"], pl["NBT"], pl["NBINS"]
    ml = __import__("ml_dtypes")
    bf16 = ml.bfloat16
    f8 = ml.float8_e4m3

    # per-edge attention bias, all layers at once: [E, L]
    A_full = ea @ np.float32(fw["wal"]).T

    M = np.float32(fw["M"])                          # [7, 256, 258]
    Mb = np.ascontiguousarray(
        M.reshape(N_LAYERS + 1, 2, 128, 258).transpose(2, 0, 1, 3)
    ).astype(bf16)                                   # [128, 7, 2, 258]
    brow = np.ascontiguousarray(
        np.broadcast_to(np.float32(fw["brow"])[None], (128, N_LAYERS + 1, 258))
    ).astype(bf16)
    W3p = np.ascontiguousarray(
        np.float32(fw["W3p"]).reshape(2, 128, OUT_DIM).transpose(1, 0, 2)
    ).astype(bf16)                                   # [128, 2, 256]

    common = dict(Mb=Mb, brow=brow, W3p=W3p)
    maps = []
    for c in range(NC):
        xs = np.zeros((DP, HID), np.float32)
        xs[:DC] = x[c * DC:(c + 1) * DC]
        m = dict(common)
        m["xT"] = np.ascontiguousarray(xs.T).astype(bf16)
        m["gidx"] = wrap_idx16(pl["gidx"][c], pl["MERGE"] * NBT * 128)
        m["nreal"] = pl["nreal"][c][None, :].astype(np.int32)
        # A in device layout [128, L, NSLOT//128]
        Ac = np.zeros((NSLOT, N_LAYERS), np.float32)
        real = pl["perm"][c] >= 0
        Ac[real] = A_full[pl["perm"][c][real]]
        m["A"] = np.ascontiguousarray(
            Ac.reshape(NSLOT // 128, 128, N_LAYERS).transpose(1, 2, 0)
        ).astype(bf16)
        # per-bin transposed onehot [slot_in_bin(p), dst_col], f8
        oh = pl["oneh"][c]
        oh_se = np.zeros((128, NSLOT), np.int8)
        for b in range(NBINS):
            oh_se[:, b * 128:(b + 1) * 128] = oh[:, b * 128:(b + 1) * 128].T
        m["oneh"] = oh_se.astype(f8)
        maps.append(m)
    return maps


# ---------------- numpy emulation (plan/fold validation) ----------------
def emulate(cfg, inputs, pl, fw):
    x = np.asarray(inputs["x"], np.float32)
    ea = np.asarray(inputs["edge_attr"], np.float32)
    DC, DP, HALF, TBL, NG = (cfg["D_CORE"], cfg["D_PAD"], cfg["HALF"],
                             cfg["TBL"], cfg["NG"])
    NSLOT, NBT = pl["NSLOT"], pl["NBT"]
    M = np.float32(fw["M"])
    brow = np.float32(fw["brow"])
    W3p = np.float32(fw["W3p"])

    A_full = ea @ np.float32(fw["wal"]).T
    A = np.zeros((NC, NSLOT, N_LAYERS), np.float32)
    for c in range(NC):
        real = pl["perm"][c] >= 0
        A[c][real] = A_full[pl["perm"][c][real]]

    # h0 phase
    mt = np.zeros((NC, DP, 257), np.float32)
    for c in range(NC):
        xs = np.zeros((DP, HID), np.float32)
        xs[:DC] = x[c * DC:(c + 1) * DC]
        mt[c] = xs @ M[0, :, 0:257] + brow[0, 0:257]

    slot_g = (np.arange(NSLOT) // (128 * NBT)) % NG
    out = np.zeros((NC, DP, OUT_DIM), np.float32)
    for i in range(N_LAYERS):
        # tables from mt
        agin = mt[:, :, 0:HID]
        sdst = mt[:, :, HID]
        AR, BR = cfg["AR"], cfg["BR"]
        AR0, BR0 = cfg["AR0"], cfg["BR0"]
        T0 = np.concatenate(
            [agin[:, :AR0].reshape(NC * AR0, HID),
             agin[:, AR0:HALF].reshape(NC * BR0, HID)], 0)
        T1 = np.concatenate(
            [agin[:, HALF:HALF + AR].reshape(NC * AR, HID),
             agin[:, HALF + AR:].reshape(NC * BR, HID)], 0)
        mt2 = np.zeros((NC, DP, 257), np.float32)
        for c in range(NC):
            gi = pl["gidx"][c].astype(np.int64)
            valid = pl["perm"][c] >= 0
            slot_t = np.arange(NSLOT) // (NG * NBT * 128)
            G = np.zeros((NSLOT, HID), np.float32)
            G[valid & (slot_t == 0)] = T0[gi[valid & (slot_t == 0)]]
            G[valid & (slot_t == 1)] = T1[gi[valid & (slot_t == 1)]]
            ssrc = G[:, HID - 1]
            oh = pl["oneh"][c].astype(np.float32)    # [dst_local, slot]
            sdsel = np.zeros(NSLOT, np.float32)
            for gg in range(NG):
                sl = slot_g == gg
                sdsel[sl] = oh[:, sl].T @ sdst[c, gg * 128:(gg + 1) * 128]
            alpha = ssrc + sdsel + A[c, :, i]
            eac = np.maximum(np.exp(alpha), np.exp(NEG_SLOPE * alpha))
            U = np.zeros((DP, HID), np.float32)
            dns = np.zeros(DP, np.float32)
            Se = oh * eac[None, :]
            for gg in range(NG):
                sl = slot_g == gg
                U[gg * 128:(gg + 1) * 128] = Se[:, sl] @ G[sl]
                dns[gg * 128:(gg + 1) * 128] = Se[:, sl].sum(1)
            U = U / (dns + EPS)[:, None]
            if i < N_LAYERS - 1:
                mt2[c] = U @ M[1 + i, :, 0:257] + brow[1 + i, 0:257]
                mt2[c, DC:] = 0.0
            else:
                h7 = U @ M[1 + i, :, 0:HID] + brow[1 + i, 0:HID]
                out[c] = np.maximum(h7, 0.0) @ W3p
        mt = mt2
    return np.concatenate([out[c, :DC] for c in range(NC)], 0)


# ---------------- device kernel ----------------
def build(cfg, pl, queues=4, debug_taps=False, zero_g=False, no_ag=False,
          gq=4):
    import concourse.bass as bass
    import concourse.tile as tile
    import concourse.mybir as mybir
    from concourse import bacc
    from concourse.masks import make_identity

    f32, bf16, i16, i32 = (mybir.dt.float32, mybir.dt.bfloat16,
                           mybir.dt.int16, mybir.dt.int32)
    f8 = mybir.dt.float8e4
    ACT = mybir.ActivationFunctionType
    ALU = mybir.AluOpType

    DP, HALF, TBL, NG = cfg["D_PAD"], cfg["HALF"], cfg["TBL"], cfg["NG"]
    G2, AR = cfg["G2"], cfg["AR"]
    G1, AR0 = cfg["G1"], cfg["AR0"]
    NBT, NSLOT, NCALLS = pl["NBT"], pl["NSLOT"], pl["NCALLS"]
    CL = NBT * 128
    NKC = HID // 128
    NJ = NSLOT // 128

    nc = bacc.Bacc(None, target_bir_lowering=False, debug=False,
                   num_swdge_queues=queues)

    # inputs
    xT = nc.dram_tensor("xT", [HID, DP], bf16, kind="ExternalInput")
    gidxD = nc.dram_tensor("gidx", [128, NSLOT // 16], i16, kind="ExternalInput")
    nrealD = nc.dram_tensor("nreal", [1, NCALLS], i32, kind="ExternalInput")
    onehD = nc.dram_tensor("oneh", [128, NSLOT], f8, kind="ExternalInput")
    AD = nc.dram_tensor("A", [128, N_LAYERS, NJ], bf16, kind="ExternalInput")
    MbD = nc.dram_tensor("Mb", [128, N_LAYERS + 1, NKC, 258], bf16,
                         kind="ExternalInput")
    browD = nc.dram_tensor("brow", [128, N_LAYERS + 1, 258], bf16,
                           kind="ExternalInput")
    W3pD = nc.dram_tensor("W3p", [128, NKC, OUT_DIM], bf16,
                          kind="ExternalInput")
    outD = nc.dram_tensor("out", [DP, OUT_DIM], f32, kind="ExternalOutput")
    dbg = {}
    if debug_taps:
        for nm, shp, dt in [("dbg_T0", [TBL, HID], bf16),
                            ("dbg_srep", [128, 128], bf16),
                            ("dbg_sc", [128, 16 * NBT], f32),
                            ("dbg_G", [128, NBT * HID], bf16),
                            ("dbg_Se", [128, 2 * NBT * 128], bf16),
                            ("dbg_gps", [128, 258], f32),
                            ("dbg_hn", [128, HID], bf16),
                            ("dbg_mt", [128, 258], f32)]:
            dbg[nm] = nc.dram_tensor(nm, shp, dt, kind="ExternalOutput")

    # internals (ping-pong tables/agin by layer parity)
    aginD = [nc.dram_tensor(f"agin{p}", [DP, HID], bf16) for p in (0, 1)]
    T0D = [nc.dram_tensor(f"T0_{p}", [TBL, HID], bf16, addr_space="Shared")
           for p in (0, 1)]
    T1D = [nc.dram_tensor(f"T1_{p}", [TBL, HID], bf16, addr_space="Shared")
           for p in (0, 1)]
    sdTD = [nc.dram_tensor(f"sdT{p}", [NG, 128], bf16) for p in (0, 1)]

    rg = [list(range(NC))]

    with tile.TileContext(nc) as tc:
        with (
            tc.tile_pool(name="res", bufs=1) as res,
            tc.tile_pool(name="lw", bufs=4) as lw,
            tc.tile_pool(name="gp", bufs=10) as gp,
            tc.tile_pool(name="sep", bufs=6) as sep,
            tc.tile_pool(name="exm", bufs=4) as exmp,
            tc.tile_pool(name="sc", bufs=6) as scp,
            tc.tile_pool(name="hn", bufs=5) as hnp,
            tc.tile_pool(name="hT", bufs=5) as hTp,
            tc.tile_pool(name="hex", bufs=5) as hex_,
            tc.tile_pool(name="acc", bufs=4, space="PSUM") as accp,
            tc.tile_pool(name="dns", bufs=2, space="PSUM") as dnsp,
            tc.tile_pool(name="tpp", bufs=2, space="PSUM") as tpp,
        ):
            # resident inputs
            gidx_sb = res.tile([128, NSLOT // 16], i16)
            nc.sync.dma_start(gidx_sb[:], gidxD[:])
            nreal_sb = res.tile([1, NCALLS], i32)
            nc.sync.dma_start(nreal_sb[:], nrealD[:])
            oneh_sb = res.tile([128, NSLOT], f8)
            nc.sync.dma_start(oneh_sb[:], onehD[:])
            A_sb = res.tile([128, N_LAYERS, NJ], bf16)
            nc.sync.dma_start(A_sb[:], AD[:])
            Mb_sb = res.tile([128, N_LAYERS + 1, NKC, 258], bf16)
            nc.sync.dma_start(Mb_sb[:], MbD[:])
            brow_sb = res.tile([128, N_LAYERS + 1, 258], bf16)
            nc.sync.dma_start(brow_sb[:], browD[:])
            W3p_sb = res.tile([128, NKC, OUT_DIM], bf16)
            nc.sync.dma_start(W3p_sb[:], W3pD[:])

            ident = res.tile([128, 128], bf16)
            make_identity(nc, ident[:])
            ones_col = res.tile([128, 1], bf16)
            nc.vector.memset(ones_col[:], 1.0)
            sdst_bf = res.tile([128, NG], bf16)
            nreal_reg = nc.gpsimd.alloc_register("nreal_reg")

            def zero_pad_suffix(G, call, nbins):
                """Sim-only: zero pad slots (logical tiles are NaN there).
                On HW the pool priming below keeps stale pads finite, which
                is all the masked (oneh=0) reads need."""
                if not zero_g:
                    return
                b0 = int(pl["minr"][call]) // 128
                if b0 < nbins:
                    nc.vector.memset(
                        G[:, b0:nbins, :].rearrange("p a b -> p (a b)"), 0.0)
            if not zero_g:
                for _ in range(10):
                    gt = gp.tile([128, pl["MERGE"] * NBT, HID], bf16, tag="G",
                                 name="gprime")
                    nc.vector.memset(gt[:].rearrange("p a b -> p (a b)"), 0.0)

            def mm_retire(src_sb, li, g, wr_parity):
                """matmul src^T @ M[li] (+brow) -> table row + sdst col."""
                mt = accp.tile([128, 258], f32, tag="acc", name="mt")
                for kc in range(NKC):
                    nc.tensor.matmul(mt[:, 0:257], src_sb[:, kc, :],
                                     Mb_sb[:, li, kc, 0:257],
                                     start=(kc == 0), stop=(kc == NKC - 1))
                hx = hex_.tile([128, HID], bf16, tag="hx")
                nc.vector.tensor_tensor(out=hx[:], in0=mt[:, 0:HID],
                                        in1=brow_sb[:, li, 0:HID], op=ALU.add)
                nc.vector.tensor_tensor(out=sdst_bf[:, g:g + 1],
                                        in0=mt[:, HID:HID + 1],
                                        in1=brow_sb[:, li, HID:HID + 1],
                                        op=ALU.add)
                nc.sync.dma_start(aginD[wr_parity][g * 128:(g + 1) * 128, :],
                                  hx[:])

            def emit_ags(g, wr_parity):
                if no_ag:
                    return
                if g == G1 - 1:
                    nc.gpsimd.collective_compute(
                        "AllGather", ALU.bypass, replica_groups=rg,
                        ins=[aginD[wr_parity][0:AR0, :]],
                        outs=[T0D[wr_parity][0:NC * AR0, :]])
                if g == NG // 2 - 1:
                    nc.gpsimd.collective_compute(
                        "AllGather", ALU.bypass, replica_groups=rg,
                        ins=[aginD[wr_parity][AR0:HALF, :]],
                        outs=[T0D[wr_parity][NC * AR0:TBL, :]])
                if g == G2 - 1:
                    nc.gpsimd.collective_compute(
                        "AllGather", ALU.bypass, replica_groups=rg,
                        ins=[aginD[wr_parity][HALF:HALF + AR, :]],
                        outs=[T1D[wr_parity][0:NC * AR, :]])
                if g == NG - 1:
                    nc.gpsimd.collective_compute(
                        "AllGather", ALU.bypass, replica_groups=rg,
                        ins=[aginD[wr_parity][HALF + AR:DP, :]],
                        outs=[T1D[wr_parity][NC * AR:TBL, :]])

            def sdst_transpose(parity):
                sdT_ps = tpp.tile([128, 128], bf16, tag="tp", name="sdT_ps")
                nc.tensor.transpose(sdT_ps[0:NG, :], sdst_bf[:], ident[:])
                sdT = hTp.tile([128, 128], bf16, tag="sdT", name="sdT")
                nc.vector.tensor_copy(sdT[0:NG, :], sdT_ps[0:NG, :])
                nc.sync.dma_start(sdTD[parity][:], sdT[0:NG, :])

            # ---------- h0: T^(0) = x @ W1W2R0 ----------
            with nc.named_scope("h0"):
                for g in range(NG):
                    xt = lw.tile([128, NKC, 128], bf16, tag="xt")
                    for kc in range(NKC):
                        nc.sync.dma_start(
                            xt[:, kc, :],
                            xT[kc * 128:(kc + 1) * 128, g * 128:(g + 1) * 128])
                    mm_retire(xt, 0, g, 0)
                    emit_ags(g, 0)
                sdst_transpose(0)

            # ---------- layers ----------
            MERGE, NJ = pl["MERGE"], pl["NJ"]
            CLM = MERGE * CL
            for i in range(N_LAYERS):
                last = i == N_LAYERS - 1
                rd, wr = i % 2, (i + 1) % 2
                if debug_taps and i == 0:
                    nc.sync.dma_start(dbg["dbg_T0"][:], T0D[0][:])
                with nc.named_scope(f"eg{i}"):
                    for j in range(NJ):
                        Gm = [None, None]
                        for t in (0, 1):
                            call = t * NJ + j
                            G = gp.tile([128, MERGE * NBT, HID], bf16,
                                        tag="G")
                            zero_pad_suffix(G, call, MERGE * NBT)
                            nc.gpsimd.reg_load(nreal_reg,
                                               nreal_sb[0:1, call:call + 1])
                            nc.gpsimd.dma_gather(
                                out_ap=G[:],
                                in_ap=(T0D[rd][:] if t == 0 else T1D[rd][:]),
                                idxs_ap=gidx_sb[:, call * (CLM // 16):
                                                (call + 1) * (CLM // 16)],
                                num_idxs=CLM, num_idxs_reg=nreal_reg,
                                elem_size=HID,
                                queue_num=(t * 2 + (j % 2)) % gq)
                            Gm[t] = G
                        for gsub in range(MERGE):
                            g = j * MERGE + gsub
                            dbg_this = debug_taps and i == 0 and g == 0
                            srep = lw.tile([128, 128], bf16, tag="srep",
                                           name="srep")
                            nc.sync.dma_start(
                                srep[:],
                                sdTD[rd][g:g + 1, :].to_broadcast((128, 128)))
                            if dbg_this:
                                nc.sync.dma_start(dbg["dbg_srep"][:], srep[:])
                            gps = accp.tile([128, 258], f32, tag="acc",
                                            name="gps")
                            dns = dnsp.tile([128, 1], f32, tag="dns",
                                            name="dns")
                            for t in (0, 1):
                                bb = (t * NG + g) * NBT
                                Gv = Gm[t][:, gsub * NBT:(gsub + 1) * NBT, :]
                                oh_v = oneh_sb[:, bb * 128:(bb + NBT) * 128] \
                                    .rearrange("p (a b) -> p a b", b=128)
                                exm = exmp.tile([128, NBT, 128], bf16,
                                                tag="exm")
                                nc.vector.tensor_tensor(
                                    out=exm[:], in0=oh_v,
                                    in1=srep[:, None, :]
                                    .to_broadcast([128, NBT, 128]),
                                    op=ALU.mult)
                                ex = scp.tile([128, NBT], f32, tag=f"ex{t}")
                                nc.vector.tensor_reduce(
                                    ex[:], exm[:], axis=mybir.AxisListType.X,
                                    op=ALU.add)
                                beta = scp.tile([128, NBT], f32,
                                                tag=f"beta{t}")
                                nc.vector.tensor_tensor(
                                    out=beta[:],
                                    in0=A_sb[:, i, bb:bb + NBT],
                                    in1=Gv[:, :, HID - 1], op=ALU.add)
                                alpha = scp.tile([128, NBT], f32,
                                                 tag=f"alpha{t}")
                                nc.vector.tensor_tensor(
                                    out=alpha[:], in0=beta[:], in1=ex[:],
                                    op=ALU.add)
                                e1 = scp.tile([128, NBT], f32, tag=f"e1{t}")
                                nc.scalar.activation(e1[:], alpha[:], ACT.Exp)
                                e2 = scp.tile([128, NBT], f32, tag=f"e2{t}")
                                nc.scalar.activation(e2[:], alpha[:], ACT.Exp,
                                                     scale=NEG_SLOPE)
                                eac = scp.tile([128, NBT], f32, tag=f"eac{t}")
                                nc.vector.tensor_tensor(
                                    out=eac[:], in0=e1[:], in1=e2[:],
                                    op=ALU.max)
                                Se = sep.tile([128, NBT, 128], bf16,
                                              tag=f"Se{t}")
                                for b in range(NBT):
                                    nc.scalar.activation(
                                        Se[:, b, :], oh_v[:, b, :],
                                        ACT.Copy, scale=eac[:, b:b + 1])
                                if dbg_this:
                                    sc = dbg["dbg_sc"]
                                    nc.sync.dma_start(
                                        sc[:, (0 + t) * NBT:(1 + t) * NBT],
                                        ex[:])
                                    nc.sync.dma_start(
                                        sc[:, (2 + t) * NBT:(3 + t) * NBT],
                                        beta[:])
                                    nc.sync.dma_start(
                                        sc[:, (4 + t) * NBT:(5 + t) * NBT],
                                        alpha[:])
                                    nc.sync.dma_start(
                                        sc[:, (6 + t) * NBT:(7 + t) * NBT],
                                        eac[:])
                                    nc.sync.dma_start(
                                        dbg["dbg_Se"][:, t * NBT * 128:
                                                      (t + 1) * NBT * 128],
                                        Se[:].rearrange("p a b -> p (a b)"))
                                    if t == 0:
                                        nc.sync.dma_start(
                                            dbg["dbg_G"][:],
                                            Gv[:].rearrange(
                                                "p a b -> p (a b)"))
                                for b in range(NBT):
                                    ii = t * NBT + b
                                    nc.tensor.matmul(
                                        gps[:, 0:HID], Se[:, b, :],
                                        Gv[:, b, :],
                                        start=(ii == 0),
                                        stop=(ii == 2 * NBT - 1))
                                    nc.tensor.matmul(
                                        dns[:], Se[:, b, :], ones_col[:],
                                        start=(ii == 0),
                                        stop=(ii == 2 * NBT - 1))
                            dcol = scp.tile([128, 1], f32, tag="dcol")
                            nc.vector.tensor_scalar_add(dcol[:], dns[:], EPS)
                            rcol = scp.tile([128, 1], f32, tag="rcol")
                            nc.vector.reciprocal(rcol[:], dcol[:])
                            hn = hnp.tile([128, HID], bf16, tag="hn")
                            if dbg_this:
                                gcp = hex_.tile([128, 258], f32, tag="gcp",
                                                name="gcp")
                                nc.vector.tensor_copy(gcp[:, 0:HID],
                                                      gps[:, 0:HID])
                                nc.vector.tensor_copy(gcp[:, HID:HID + 1],
                                                      dns[:])
                                nc.sync.dma_start(dbg["dbg_gps"][:], gcp[:])
                            nc.scalar.activation(hn[:], gps[:, 0:HID],
                                                 ACT.Copy, scale=rcol[:, 0:1])
                            if dbg_this:
                                nc.sync.dma_start(dbg["dbg_hn"][:], hn[:])
                            tp = tpp.tile([128, NKC, 128], bf16, tag="tp",
                                          name="tp")
                            for kc in range(NKC):
                                nc.tensor.transpose(
                                    tp[:, kc, :],
                                    hn[:, kc * 128:(kc + 1) * 128], ident[:])
                            hT = hTp.tile([128, NKC, 128], bf16, tag="hT")
                            nc.vector.tensor_copy(hT[:], tp[:])
                            if not last:
                                mm_retire(hT, 1 + i, g, wr)
                                emit_ags(g, wr)
                            else:
                                mt = accp.tile([128, 258], f32, tag="acc",
                                               name="mt6")
                                for kc in range(NKC):
                                    nc.tensor.matmul(
                                        mt[:, 0:HID], hT[:, kc, :],
                                        Mb_sb[:, 1 + i, kc, 0:HID],
                                        start=(kc == 0), stop=(kc == NKC - 1))
                                h6x = hex_.tile([128, HID], f32, tag="h6x",
                                                name="h6x")
                                nc.vector.tensor_tensor(
                                    out=h6x[:], in0=mt[:, 0:HID],
                                    in1=brow_sb[:, 1 + i, 0:HID], op=ALU.add)
                                rh = hnp.tile([128, HID], bf16, tag="rh",
                                              name="rh")
                                nc.scalar.activation(rh[:], h6x[:], ACT.Relu)
                                tp2 = tpp.tile([128, NKC, 128], bf16,
                                               tag="tp", name="tp2")
                                for kc in range(NKC):
                                    nc.tensor.transpose(
                                        tp2[:, kc, :],
                                        rh[:, kc * 128:(kc + 1) * 128],
                                        ident[:])
                                rhT = hTp.tile([128, NKC, 128], bf16,
                                               tag="hT", name="rhT")
                                nc.vector.tensor_copy(rhT[:], tp2[:])
                                ops = accp.tile([128, 258], f32, tag="acc",
                                                name="ops")
                                for kc in range(NKC):
                                    nc.tensor.matmul(
                                        ops[:, 0:OUT_DIM], rhT[:, kc, :],
                                        W3p_sb[:, kc, :],
                                        start=(kc == 0), stop=(kc == NKC - 1))
                                outf = hex_.tile([128, OUT_DIM], f32,
                                                 tag="outf", name="outf")
                                nc.vector.tensor_copy(outf[:],
                                                      ops[:, 0:OUT_DIM])
                                nc.sync.dma_start(
                                    outD[g * 128:(g + 1) * 128, :], outf[:])
                    if not last:
                        sdst_transpose(wr)

    nc.compile()
    return nc


_CACHE = {}


def kernel(**inputs) -> np.ndarray:
    from concourse.bass_utils import run_bass_kernel_spmd

    cfg = make_cfg()
    ei = np.asarray(inputs["edge_index"])
    pl = plan(cfg, ei)
    key = ("nc", pl["NBT"])
    if key not in _CACHE:
        _CACHE[key] = build(cfg, pl)
    nc = _CACHE[key]
    fw = fold_weights(inputs)
    maps = prep_inputs(cfg, pl, fw, inputs)
    res = run_bass_kernel_spmd(nc, maps, core_ids=list(range(NC)))
    DC = cfg["D_CORE"]
    return np.concatenate([res.results[c]["out"][:DC] for c in range(NC)],
                          0).astype(np.float32)



# revision 32
# speedup vs baseline: 1.0505x; 1.0254x over previous
"""GAT GNN (edge features) Trainium2 kernel — 8-core SPMD, v2.

Sharding: nodes by dst range (6250/core, padded 6400). Table rows are 512B
(256 bf16): per-layer features are kept in a rotated basis h@R_l whose last
column equals Wc_l@att_src_l, so the per-edge s_src logit is just column 255
of the gathered row (no extra embedded scalar -> 512B rows, 33% less gather
and AllGather traffic than 768B). All weight algebra (R_l^{-1} Wc_l R_{l+1}
folds, s_dst columns, We@att_edge projections) is folded on the host; the
per-edge attention bias A = edge_attr @ (We_l att_edge_l) is computed on the
host as well (it is layer-input independent).

Per layer: gather 512B rows (num_idxs_reg-trimmed) -> logits (A + G[:,255]
+ onehot-expanded s_dst via DVE mult+reduce) -> exp/leaky-relu -> Se ->
PSUM numerator+denominator (denominator as column 256 of the same PSUM
tile) -> normalize -> PE-transpose -> fused mm (R^-1 Wc R fold + s_dst
column) -> AllGather into parity ping-pong tables (overlaps next groups).
"""
import sys

sys.path.insert(0, "/opt/trn_rl_repo")

import numpy as np

NEG_SLOPE = 0.2
EPS = 1e-16
NC = 8
HID = 256
EDGE_DIM = 768
OUT_DIM = 256
N_LAYERS = 6


def make_cfg(n_nodes=50000, n_edges=400000):
    c = {}
    c["N"] = n_nodes
    c["E"] = n_edges
    c["D_CORE"] = n_nodes // NC
    c["D_PAD"] = -(-c["D_CORE"] // 128) * 128
    if (c["D_PAD"] // 128) % 2:
        c["D_PAD"] += 128          # even group count so HALF is 128-aligned
    c["HALF"] = c["D_PAD"] // 2
    c["TBL"] = NC * c["HALF"]
    assert c["TBL"] < 32768
    c["NG"] = c["D_PAD"] // 128
    # Each table half is AllGathered in two contiguous pieces so the
    # collective pipeline starts earlier and only the small tail piece
    # (groups G2..NG-1) is exposed at the layer boundary.  Row layout of
    # each half is piece-major (piece A's 8 cores, then piece B's).
    c["G1"] = 13                                      # T0 piece-A groups
    c["AR0"] = c["G1"] * 128
    c["BR0"] = c["HALF"] - c["AR0"]
    # T1 in three pieces (10/8/7 groups) so the collective queue drains
    # and only the last ~7-group piece is exposed at the layer boundary.
    c["G2a"] = (c["NG"] // 2) + 10
    c["G2b"] = (c["NG"] // 2) + 18
    c["AR1"] = 10 * 128                               # T1 piece-A rows/core
    c["AR2"] = 8 * 128                                # T1 piece-B rows/core
    c["BR"] = c["D_PAD"] - c["HALF"] - c["AR1"] - c["AR2"]   # piece-C
    return c


# ---------------- host planner ----------------
def plan(cfg, edge_index, merge=1):
    """Slot space is t-major: slot = ((t*NG + g)*NBT + b)*128 + p.  Gather
    calls cover `merge` consecutive groups of one table half; only the last
    group's trailing pad is trimmed (middle pads gather row 0 harmlessly)."""
    src = np.asarray(edge_index[0], np.int64)
    dst = np.asarray(edge_index[1], np.int64)
    DC, HALF, NG = cfg["D_CORE"], cfg["HALF"], cfg["NG"]
    AR1, AR2, BR = cfg["AR1"], cfg["AR2"], cfg["BR"]
    AR0, BR0 = cfg["AR0"], cfg["BR0"]
    assert NG % merge == 0

    per_core = []
    nbt = 1
    for c in range(NC):
        m = (dst >= c * DC) & (dst < (c + 1) * DC)
        eid = np.nonzero(m)[0]
        es, ed = src[eid], dst[eid] - c * DC
        et = ((es % DC) >= HALF).astype(np.int64)
        cs, ls = es // DC, es % DC
        # Both halves are piece-major (piece A's 8 cores, then piece B's),
        # core-major within each piece (matches the 2-piece AllGathers).
        erow_t0 = np.where(ls < AR0,
                           cs * AR0 + ls,
                           NC * AR0 + cs * BR0 + (ls - AR0))
        erow_t1 = np.where(
            ls < HALF + AR1,
            cs * AR1 + (ls - HALF),
            np.where(ls < HALF + AR1 + AR2,
                     NC * AR1 + cs * AR2 + (ls - HALF - AR1),
                     NC * (AR1 + AR2) + cs * BR + (ls - HALF - AR1 - AR2)))
        erow = np.where(et == 0, erow_t0, erow_t1)
        g = ed // 128
        per_core.append((eid, es, ed, et, erow, g))
        cnt = np.zeros((NG, 2), np.int64)
        np.add.at(cnt, (g, et), 1)
        nbt = max(nbt, int(-(-cnt.max() // 128)), 1)
    NBT = nbt
    NBINS = NG * 2 * NBT
    NSLOT = NBINS * 128
    NJ = NG // merge
    NCALLS = 2 * NJ

    gidx = np.full((NC, NSLOT), -1, np.int16)
    ngrp = np.zeros((NC, 2, NG), np.int32)        # real slots per (t, g)
    onehT = np.zeros((NC, 128, NSLOT), np.int8)   # [dst_local, slot]
    perm = np.full((NC, NSLOT), -1, np.int64)
    for c in range(NC):
        eid, es, ed, et, erow, g = per_core[c]
        for gg in range(NG):
            for t in (0, 1):
                sel = np.nonzero((g == gg) & (et == t))[0]
                base = ((t * NG + gg) * NBT) * 128
                ngrp[c, t, gg] = max(len(sel), 1)
                if len(sel) == 0:
                    continue
                slots = base + np.arange(len(sel))
                gidx[c, slots] = erow[sel].astype(np.int16)
                perm[c, slots] = eid[sel]
                onehT[c, ed[sel] - gg * 128, slots] = 1
    # merged-call trim counts: full middle groups + last group's real count.
    # Ucode contract: num_idxs_reg == count of idx >= 0, and only TRAILING
    # pads may be -1 -> pads below the trim point become row 0 (harmless).
    nreal = np.zeros((NC, NCALLS), np.int32)
    CLM = merge * NBT * 128
    for c in range(NC):
        for t in (0, 1):
            for j in range(NJ):
                nr = ((merge - 1) * NBT * 128
                      + ngrp[c, t, j * merge + merge - 1])
                nreal[c, t * NJ + j] = nr
                s = (t * NG + j * merge) * NBT * 128
                seg = gidx[c, s:s + nr]
                seg[seg < 0] = 0
    return dict(NBT=NBT, NBINS=NBINS, NSLOT=NSLOT, NCALLS=NCALLS,
                MERGE=merge, NJ=NJ, minr=nreal.min(axis=0),
                gidx=gidx, nreal=nreal, oneh=onehT, perm=perm)


def wrap_idx16(gidx, call_len):
    """[NSLOT] -> [128, NSLOT//16] with per-call 16-partition wrap."""
    ncalls = gidx.shape[0] // call_len
    blk = gidx.reshape(ncalls, call_len // 16, 16).transpose(2, 0, 1)
    flat = blk.reshape(16, ncalls * (call_len // 16))
    return np.tile(flat, (8, 1))


# ---------------- host weight folding ----------------
def fold_weights(inputs):
    """R_l rotations + fused per-layer rhs matrices, all in f64.

    Table basis: T^(i) = h^(i) @ R_i, with R_i[:, 255] = Wc_i @ att_src_i so
    s_src == gathered column 255.  R_i = H_i @ diag(1,..,1, beta*n) with H_i
    a Householder reflector, so R_i^{-1} is exact.
    """
    W1 = np.float64(inputs["W1"])
    W2 = np.float64(inputs["W2"])
    Wc = np.float64(inputs["Wc"])
    We = np.float64(inputs["We"])
    a_s = np.float64(inputs["att_src"])
    a_d = np.float64(inputs["att_dst"])
    a_e = np.float64(inputs["att_edge"])
    bias = np.float64(inputs["bias"])
    W3 = np.float64(inputs["W3"])

    R = []
    Rinv = []
    for i in range(N_LAYERS):
        v = Wc[i] @ a_s[i]
        n = np.linalg.norm(v)
        u = v / n
        beta = -1.0 if u[HID - 1] > 0 else 1.0
        w = u.copy()
        w[HID - 1] -= beta
        H = np.eye(HID) - 2.0 * np.outer(w, w) / (w @ w)
        # H @ e_last = beta*u  ->  R[:,255] = H[:,255] * (beta*n) = u*n = v
        Ri = H.copy()
        Ri[:, HID - 1] *= beta * n
        Rii = H.copy()                      # R^-1 = diag(1,..,1/(beta n)) @ H
        Rii[HID - 1, :] /= beta * n
        R.append(Ri)
        Rinv.append(Rii)

    v_d = [Wc[i] @ a_d[i] for i in range(N_LAYERS)]

    # mm matrices: index 0 = h0 producer (x @ W1W2 -> T^(0));
    # index 1+i = applied after layer i's aggregation.
    M = np.zeros((N_LAYERS + 1, HID, 258))
    brow = np.zeros((N_LAYERS + 1, 258))
    W12 = W1 @ W2
    M[0, :, 0:HID] = W12 @ R[0]
    M[0, :, HID] = W12 @ v_d[0]
    for i in range(N_LAYERS - 1):
        M[1 + i, :, 0:HID] = Rinv[i] @ Wc[i] @ R[i + 1]
        M[1 + i, :, HID] = Rinv[i] @ Wc[i] @ v_d[i + 1]
        brow[1 + i, 0:HID] = bias[i] @ R[i + 1]
        brow[1 + i, HID] = bias[i] @ v_d[i + 1]
    M[N_LAYERS, :, 0:HID] = Rinv[N_LAYERS - 1] @ Wc[N_LAYERS - 1]
    brow[N_LAYERS, 0:HID] = bias[N_LAYERS - 1]

    wal = np.einsum("lkh,lh->lk", We, a_e)          # [L, EDGE_DIM]
    W3p = W3[:HID] + W3[HID:]                        # [HID, OUT]
    return dict(R=R, Rinv=Rinv, M=M, brow=brow, wal=wal, W3p=W3p)


# ---------------- host-side input prep ----------------
def prep_inputs(cfg, pl, fw, inputs):
    x = np.asarray(inputs["x"], np.float32)
    ea = np.asarray(inputs["edge_attr"], np.float32)
    DC, DP = cfg["D_CORE"], cfg["D_PAD"]
    NSLOT, NBT, NBINS = pl["NSLOT"], pl["NBT"], pl["NBINS"]
    ml = __import__("ml_dtypes")
    bf16 = ml.bfloat16
    f8 = ml.float8_e4m3

    # per-edge attention bias, all layers at once: [E, L]
    A_full = ea @ np.float32(fw["wal"]).T

    M = np.float32(fw["M"])                          # [7, 256, 258]
    Mb = np.ascontiguousarray(
        M.reshape(N_LAYERS + 1, 2, 128, 258).transpose(2, 0, 1, 3)
    ).astype(bf16)                                   # [128, 7, 2, 258]
    brow = np.ascontiguousarray(
        np.broadcast_to(np.float32(fw["brow"])[None], (128, N_LAYERS + 1, 258))
    ).astype(bf16)
    W3p = np.ascontiguousarray(
        np.float32(fw["W3p"]).reshape(2, 128, OUT_DIM).transpose(1, 0, 2)
    ).astype(bf16)                                   # [128, 2, 256]

    common = dict(Mb=Mb, brow=brow, W3p=W3p)
    maps = []
    for c in range(NC):
        xs = np.zeros((DP, HID), np.float32)
        xs[:DC] = x[c * DC:(c + 1) * DC]
        m = dict(common)
        m["xT"] = np.ascontiguousarray(xs.T).astype(bf16)
        m["gidx"] = wrap_idx16(pl["gidx"][c], pl["MERGE"] * NBT * 128)
        m["nreal"] = pl["nreal"][c][None, :].astype(np.int32)
        # A in device layout [128, L, NSLOT//128]
        Ac = np.zeros((NSLOT, N_LAYERS), np.float32)
        real = pl["perm"][c] >= 0
        Ac[real] = A_full[pl["perm"][c][real]]
        m["A"] = np.ascontiguousarray(
            Ac.reshape(NSLOT // 128, 128, N_LAYERS).transpose(1, 2, 0)
        ).astype(bf16)
        # per-bin transposed onehot [slot_in_bin(p), dst_col], f8
        oh = pl["oneh"][c]
        oh_se = np.zeros((128, NSLOT), np.int8)
        for b in range(NBINS):
            oh_se[:, b * 128:(b + 1) * 128] = oh[:, b * 128:(b + 1) * 128].T
        m["oneh"] = oh_se.astype(f8)
        maps.append(m)
    return maps


# ---------------- numpy emulation (plan/fold validation) ----------------
def emulate(cfg, inputs, pl, fw):
    x = np.asarray(inputs["x"], np.float32)
    ea = np.asarray(inputs["edge_attr"], np.float32)
    DC, DP, HALF, TBL, NG = (cfg["D_CORE"], cfg["D_PAD"], cfg["HALF"],
                             cfg["TBL"], cfg["NG"])
    NSLOT, NBT = pl["NSLOT"], pl["NBT"]
    M = np.float32(fw["M"])
    brow = np.float32(fw["brow"])
    W3p = np.float32(fw["W3p"])

    A_full = ea @ np.float32(fw["wal"]).T
    A = np.zeros((NC, NSLOT, N_LAYERS), np.float32)
    for c in range(NC):
        real = pl["perm"][c] >= 0
        A[c][real] = A_full[pl["perm"][c][real]]

    # h0 phase
    mt = np.zeros((NC, DP, 257), np.float32)
    for c in range(NC):
        xs = np.zeros((DP, HID), np.float32)
        xs[:DC] = x[c * DC:(c + 1) * DC]
        mt[c] = xs @ M[0, :, 0:257] + brow[0, 0:257]

    slot_g = (np.arange(NSLOT) // (128 * NBT)) % NG
    out = np.zeros((NC, DP, OUT_DIM), np.float32)
    for i in range(N_LAYERS):
        # tables from mt
        agin = mt[:, :, 0:HID]
        sdst = mt[:, :, HID]
        AR1, AR2, BR = cfg["AR1"], cfg["AR2"], cfg["BR"]
        AR0, BR0 = cfg["AR0"], cfg["BR0"]
        T0 = np.concatenate(
            [agin[:, :AR0].reshape(NC * AR0, HID),
             agin[:, AR0:HALF].reshape(NC * BR0, HID)], 0)
        T1 = np.concatenate(
            [agin[:, HALF:HALF + AR1].reshape(NC * AR1, HID),
             agin[:, HALF + AR1:HALF + AR1 + AR2].reshape(NC * AR2, HID),
             agin[:, HALF + AR1 + AR2:].reshape(NC * BR, HID)], 0)
        mt2 = np.zeros((NC, DP, 257), np.float32)
        for c in range(NC):
            gi = pl["gidx"][c].astype(np.int64)
            valid = pl["perm"][c] >= 0
            slot_t = np.arange(NSLOT) // (NG * NBT * 128)
            G = np.zeros((NSLOT, HID), np.float32)
            G[valid & (slot_t == 0)] = T0[gi[valid & (slot_t == 0)]]
            G[valid & (slot_t == 1)] = T1[gi[valid & (slot_t == 1)]]
            ssrc = G[:, HID - 1]
            oh = pl["oneh"][c].astype(np.float32)    # [dst_local, slot]
            sdsel = np.zeros(NSLOT, np.float32)
            for gg in range(NG):
                sl = slot_g == gg
                sdsel[sl] = oh[:, sl].T @ sdst[c, gg * 128:(gg + 1) * 128]
            alpha = ssrc + sdsel + A[c, :, i]
            eac = np.maximum(np.exp(alpha), np.exp(NEG_SLOPE * alpha))
            U = np.zeros((DP, HID), np.float32)
            dns = np.zeros(DP, np.float32)
            Se = oh * eac[None, :]
            for gg in range(NG):
                sl = slot_g == gg
                U[gg * 128:(gg + 1) * 128] = Se[:, sl] @ G[sl]
                dns[gg * 128:(gg + 1) * 128] = Se[:, sl].sum(1)
            U = U / (dns + EPS)[:, None]
            if i < N_LAYERS - 1:
                mt2[c] = U @ M[1 + i, :, 0:257] + brow[1 + i, 0:257]
                mt2[c, DC:] = 0.0
            else:
                h7 = U @ M[1 + i, :, 0:HID] + brow[1 + i, 0:HID]
                out[c] = np.maximum(h7, 0.0) @ W3p
        mt = mt2
    return np.concatenate([out[c, :DC] for c in range(NC)], 0)


# ---------------- device kernel ----------------
def build(cfg, pl, queues=4, debug_taps=False, zero_g=False, no_ag=False,
          gq=4):
    import concourse.bass as bass
    import concourse.tile as tile
    import concourse.mybir as mybir
    from concourse import bacc
    from concourse.masks import make_identity

    f32, bf16, i16, i32 = (mybir.dt.float32, mybir.dt.bfloat16,
                           mybir.dt.int16, mybir.dt.int32)
    f8 = mybir.dt.float8e4
    ACT = mybir.ActivationFunctionType
    ALU = mybir.AluOpType

    DP, HALF, TBL, NG = cfg["D_PAD"], cfg["HALF"], cfg["TBL"], cfg["NG"]
    G2a, G2b = cfg["G2a"], cfg["G2b"]
    AR1, AR2 = cfg["AR1"], cfg["AR2"]
    G1, AR0 = cfg["G1"], cfg["AR0"]
    NBT, NSLOT, NCALLS = pl["NBT"], pl["NSLOT"], pl["NCALLS"]
    CL = NBT * 128
    NKC = HID // 128
    NJ = NSLOT // 128

    nc = bacc.Bacc(None, target_bir_lowering=False, debug=False,
                   num_swdge_queues=queues)

    # inputs
    xT = nc.dram_tensor("xT", [HID, DP], bf16, kind="ExternalInput")
    gidxD = nc.dram_tensor("gidx", [128, NSLOT // 16], i16, kind="ExternalInput")
    nrealD = nc.dram_tensor("nreal", [1, NCALLS], i32, kind="ExternalInput")
    onehD = nc.dram_tensor("oneh", [128, NSLOT], f8, kind="ExternalInput")
    AD = nc.dram_tensor("A", [128, N_LAYERS, NJ], bf16, kind="ExternalInput")
    MbD = nc.dram_tensor("Mb", [128, N_LAYERS + 1, NKC, 258], bf16,
                         kind="ExternalInput")
    browD = nc.dram_tensor("brow", [128, N_LAYERS + 1, 258], bf16,
                           kind="ExternalInput")
    W3pD = nc.dram_tensor("W3p", [128, NKC, OUT_DIM], bf16,
                          kind="ExternalInput")
    outD = nc.dram_tensor("out", [DP, OUT_DIM], f32, kind="ExternalOutput")
    dbg = {}
    if debug_taps:
        for nm, shp, dt in [("dbg_T0", [TBL, HID], bf16),
                            ("dbg_srep", [128, 128], bf16),
                            ("dbg_sc", [128, 16 * NBT], f32),
                            ("dbg_G", [128, NBT * HID], bf16),
                            ("dbg_Se", [128, 2 * NBT * 128], bf16),
                            ("dbg_gps", [128, 258], f32),
                            ("dbg_hn", [128, HID], bf16),
                            ("dbg_mt", [128, 258], f32)]:
            dbg[nm] = nc.dram_tensor(nm, shp, dt, kind="ExternalOutput")

    # internals (ping-pong tables/agin by layer parity)
    aginD = [nc.dram_tensor(f"agin{p}", [DP, HID], bf16) for p in (0, 1)]
    T0D = [nc.dram_tensor(f"T0_{p}", [TBL, HID], bf16, addr_space="Shared")
           for p in (0, 1)]
    T1D = [nc.dram_tensor(f"T1_{p}", [TBL, HID], bf16, addr_space="Shared")
           for p in (0, 1)]
    sdTD = [nc.dram_tensor(f"sdT{p}", [NG, 128], bf16) for p in (0, 1)]

    rg = [list(range(NC))]

    with tile.TileContext(nc) as tc:
        with (
            tc.tile_pool(name="res", bufs=1) as res,
            tc.tile_pool(name="lw", bufs=4) as lw,
            tc.tile_pool(name="gp", bufs=10) as gp,
            tc.tile_pool(name="sep", bufs=6) as sep,
            tc.tile_pool(name="exm", bufs=4) as exmp,
            tc.tile_pool(name="sc", bufs=6) as scp,
            tc.tile_pool(name="hn", bufs=5) as hnp,
            tc.tile_pool(name="hT", bufs=5) as hTp,
            tc.tile_pool(name="hex", bufs=5) as hex_,
            tc.tile_pool(name="acc", bufs=4, space="PSUM") as accp,
            tc.tile_pool(name="dns", bufs=2, space="PSUM") as dnsp,
            tc.tile_pool(name="tpp", bufs=2, space="PSUM") as tpp,
        ):
            # resident inputs
            gidx_sb = res.tile([128, NSLOT // 16], i16)
            nc.sync.dma_start(gidx_sb[:], gidxD[:])
            nreal_sb = res.tile([1, NCALLS], i32)
            nc.sync.dma_start(nreal_sb[:], nrealD[:])
            oneh_sb = res.tile([128, NSLOT], f8)
            nc.sync.dma_start(oneh_sb[:], onehD[:])
            A_sb = res.tile([128, N_LAYERS, NJ], bf16)
            nc.sync.dma_start(A_sb[:], AD[:])
            Mb_sb = res.tile([128, N_LAYERS + 1, NKC, 258], bf16)
            nc.sync.dma_start(Mb_sb[:], MbD[:])
            brow_sb = res.tile([128, N_LAYERS + 1, 258], bf16)
            nc.sync.dma_start(brow_sb[:], browD[:])
            W3p_sb = res.tile([128, NKC, OUT_DIM], bf16)
            nc.sync.dma_start(W3p_sb[:], W3pD[:])

            ident = res.tile([128, 128], bf16)
            make_identity(nc, ident[:])
            ones_col = res.tile([128, 1], bf16)
            nc.vector.memset(ones_col[:], 1.0)
            sdst_bf = res.tile([128, NG], bf16)
            nreal_reg = nc.gpsimd.alloc_register("nreal_reg")

            def zero_pad_suffix(G, call, nbins):
                """Sim-only: zero pad slots (logical tiles are NaN there).
                On HW the pool priming below keeps stale pads finite, which
                is all the masked (oneh=0) reads need."""
                if not zero_g:
                    return
                b0 = int(pl["minr"][call]) // 128
                if b0 < nbins:
                    nc.vector.memset(
                        G[:, b0:nbins, :].rearrange("p a b -> p (a b)"), 0.0)
            if not zero_g:
                for _ in range(10):
                    gt = gp.tile([128, pl["MERGE"] * NBT, HID], bf16, tag="G",
                                 name="gprime")
                    nc.vector.memset(gt[:].rearrange("p a b -> p (a b)"), 0.0)

            def mm_retire(src_sb, li, g, wr_parity):
                """matmul src^T @ M[li] (+brow) -> table row + sdst col."""
                mt = accp.tile([128, 258], f32, tag="acc", name="mt")
                for kc in range(NKC):
                    nc.tensor.matmul(mt[:, 0:257], src_sb[:, kc, :],
                                     Mb_sb[:, li, kc, 0:257],
                                     start=(kc == 0), stop=(kc == NKC - 1))
                hx = hex_.tile([128, HID], bf16, tag="hx")
                nc.vector.tensor_tensor(out=hx[:], in0=mt[:, 0:HID],
                                        in1=brow_sb[:, li, 0:HID], op=ALU.add)
                nc.vector.tensor_tensor(out=sdst_bf[:, g:g + 1],
                                        in0=mt[:, HID:HID + 1],
                                        in1=brow_sb[:, li, HID:HID + 1],
                                        op=ALU.add)
                nc.sync.dma_start(aginD[wr_parity][g * 128:(g + 1) * 128, :],
                                  hx[:])

            def emit_ags(g, wr_parity):
                if no_ag:
                    return
                if g == G1 - 1:
                    nc.gpsimd.collective_compute(
                        "AllGather", ALU.bypass, replica_groups=rg,
                        ins=[aginD[wr_parity][0:AR0, :]],
                        outs=[T0D[wr_parity][0:NC * AR0, :]])
                if g == NG // 2 - 1:
                    nc.gpsimd.collective_compute(
                        "AllGather", ALU.bypass, replica_groups=rg,
                        ins=[aginD[wr_parity][AR0:HALF, :]],
                        outs=[T0D[wr_parity][NC * AR0:TBL, :]])
                if g == G2a - 1:
                    nc.gpsimd.collective_compute(
                        "AllGather", ALU.bypass, replica_groups=rg,
                        ins=[aginD[wr_parity][HALF:HALF + AR1, :]],
                        outs=[T1D[wr_parity][0:NC * AR1, :]])
                if g == G2b - 1:
                    nc.gpsimd.collective_compute(
                        "AllGather", ALU.bypass, replica_groups=rg,
                        ins=[aginD[wr_parity][HALF + AR1:HALF + AR1 + AR2, :]],
                        outs=[T1D[wr_parity][NC * AR1:NC * (AR1 + AR2), :]])
                if g == NG - 1:
                    nc.gpsimd.collective_compute(
                        "AllGather", ALU.bypass, replica_groups=rg,
                        ins=[aginD[wr_parity][HALF + AR1 + AR2:DP, :]],
                        outs=[T1D[wr_parity][NC * (AR1 + AR2):TBL, :]])

            def sdst_transpose(parity):
                sdT_ps = tpp.tile([128, 128], bf16, tag="tp", name="sdT_ps")
                nc.tensor.transpose(sdT_ps[0:NG, :], sdst_bf[:], ident[:])
                sdT = hTp.tile([128, 128], bf16, tag="sdT", name="sdT")
                nc.vector.tensor_copy(sdT[0:NG, :], sdT_ps[0:NG, :])
                nc.sync.dma_start(sdTD[parity][:], sdT[0:NG, :])

            # ---------- h0: T^(0) = x @ W1W2R0 ----------
            with nc.named_scope("h0"):
                for g in range(NG):
                    xt = lw.tile([128, NKC, 128], bf16, tag="xt")
                    for kc in range(NKC):
                        nc.sync.dma_start(
                            xt[:, kc, :],
                            xT[kc * 128:(kc + 1) * 128, g * 128:(g + 1) * 128])
                    mm_retire(xt, 0, g, 0)
                    emit_ags(g, 0)
                sdst_transpose(0)

            # ---------- layers ----------
            MERGE, NJ = pl["MERGE"], pl["NJ"]
            CLM = MERGE * CL
            for i in range(N_LAYERS):
                last = i == N_LAYERS - 1
                rd, wr = i % 2, (i + 1) % 2
                if debug_taps and i == 0:
                    nc.sync.dma_start(dbg["dbg_T0"][:], T0D[0][:])
                with nc.named_scope(f"eg{i}"):
                    for j in range(NJ):
                        Gm = [None, None]
                        for t in (0, 1):
                            call = t * NJ + j
                            G = gp.tile([128, MERGE * NBT, HID], bf16,
                                        tag="G")
                            zero_pad_suffix(G, call, MERGE * NBT)
                            nc.gpsimd.reg_load(nreal_reg,
                                               nreal_sb[0:1, call:call + 1])
                            nc.gpsimd.dma_gather(
                                out_ap=G[:],
                                in_ap=(T0D[rd][:] if t == 0 else T1D[rd][:]),
                                idxs_ap=gidx_sb[:, call * (CLM // 16):
                                                (call + 1) * (CLM // 16)],
                                num_idxs=CLM, num_idxs_reg=nreal_reg,
                                elem_size=HID,
                                queue_num=(t * 2 + (j % 2)) % gq)
                            Gm[t] = G
                        for gsub in range(MERGE):
                            g = j * MERGE + gsub
                            dbg_this = debug_taps and i == 0 and g == 0
                            srep = lw.tile([128, 128], bf16, tag="srep",
                                           name="srep")
                            nc.sync.dma_start(
                                srep[:],
                                sdTD[rd][g:g + 1, :].to_broadcast((128, 128)))
                            if dbg_this:
                                nc.sync.dma_start(dbg["dbg_srep"][:], srep[:])
                            gps = accp.tile([128, 258], f32, tag="acc",
                                            name="gps")
                            dns = dnsp.tile([128, 1], f32, tag="dns",
                                            name="dns")
                            for t in (0, 1):
                                bb = (t * NG + g) * NBT
                                Gv = Gm[t][:, gsub * NBT:(gsub + 1) * NBT, :]
                                oh_v = oneh_sb[:, bb * 128:(bb + NBT) * 128] \
                                    .rearrange("p (a b) -> p a b", b=128)
                                exm = exmp.tile([128, NBT, 128], bf16,
                                                tag="exm")
                                nc.vector.tensor_tensor(
                                    out=exm[:], in0=oh_v,
                                    in1=srep[:, None, :]
                                    .to_broadcast([128, NBT, 128]),
                                    op=ALU.mult)
                                ex = scp.tile([128, NBT], f32, tag=f"ex{t}")
                                nc.vector.tensor_reduce(
                                    ex[:], exm[:], axis=mybir.AxisListType.X,
                                    op=ALU.add)
                                beta = scp.tile([128, NBT], f32,
                                                tag=f"beta{t}")
                                nc.vector.tensor_tensor(
                                    out=beta[:],
                                    in0=A_sb[:, i, bb:bb + NBT],
                                    in1=Gv[:, :, HID - 1], op=ALU.add)
                                alpha = scp.tile([128, NBT], f32,
                                                 tag=f"alpha{t}")
                                nc.vector.tensor_tensor(
                                    out=alpha[:], in0=beta[:], in1=ex[:],
                                    op=ALU.add)
                                e1 = scp.tile([128, NBT], f32, tag=f"e1{t}")
                                nc.scalar.activation(e1[:], alpha[:], ACT.Exp)
                                e2 = scp.tile([128, NBT], f32, tag=f"e2{t}")
                                nc.scalar.activation(e2[:], alpha[:], ACT.Exp,
                                                     scale=NEG_SLOPE)
                                eac = scp.tile([128, NBT], f32, tag=f"eac{t}")
                                nc.vector.tensor_tensor(
                                    out=eac[:], in0=e1[:], in1=e2[:],
                                    op=ALU.max)
                                Se = sep.tile([128, NBT, 128], bf16,
                                              tag=f"Se{t}")
                                for b in range(NBT):
                                    nc.scalar.activation(
                                        Se[:, b, :], oh_v[:, b, :],
                                        ACT.Copy, scale=eac[:, b:b + 1])
                                if dbg_this:
                                    sc = dbg["dbg_sc"]
                                    nc.sync.dma_start(
                                        sc[:, (0 + t) * NBT:(1 + t) * NBT],
                                        ex[:])
                                    nc.sync.dma_start(
                                        sc[:, (2 + t) * NBT:(3 + t) * NBT],
                                        beta[:])
                                    nc.sync.dma_start(
                                        sc[:, (4 + t) * NBT:(5 + t) * NBT],
                                        alpha[:])
                                    nc.sync.dma_start(
                                        sc[:, (6 + t) * NBT:(7 + t) * NBT],
                                        eac[:])
                                    nc.sync.dma_start(
                                        dbg["dbg_Se"][:, t * NBT * 128:
                                                      (t + 1) * NBT * 128],
                                        Se[:].rearrange("p a b -> p (a b)"))
                                    if t == 0:
                                        nc.sync.dma_start(
                                            dbg["dbg_G"][:],
                                            Gv[:].rearrange(
                                                "p a b -> p (a b)"))
                                for b in range(NBT):
                                    ii = t * NBT + b
                                    nc.tensor.matmul(
                                        gps[:, 0:HID], Se[:, b, :],
                                        Gv[:, b, :],
                                        start=(ii == 0),
                                        stop=(ii == 2 * NBT - 1))
                                    nc.tensor.matmul(
                                        dns[:], Se[:, b, :], ones_col[:],
                                        start=(ii == 0),
                                        stop=(ii == 2 * NBT - 1))
                            dcol = scp.tile([128, 1], f32, tag="dcol")
                            nc.vector.tensor_scalar_add(dcol[:], dns[:], EPS)
                            rcol = scp.tile([128, 1], f32, tag="rcol")
                            nc.vector.reciprocal(rcol[:], dcol[:])
                            hn = hnp.tile([128, HID], bf16, tag="hn")
                            if dbg_this:
                                gcp = hex_.tile([128, 258], f32, tag="gcp",
                                                name="gcp")
                                nc.vector.tensor_copy(gcp[:, 0:HID],
                                                      gps[:, 0:HID])
                                nc.vector.tensor_copy(gcp[:, HID:HID + 1],
                                                      dns[:])
                                nc.sync.dma_start(dbg["dbg_gps"][:], gcp[:])
                            nc.scalar.activation(hn[:], gps[:, 0:HID],
                                                 ACT.Copy, scale=rcol[:, 0:1])
                            if dbg_this:
                                nc.sync.dma_start(dbg["dbg_hn"][:], hn[:])
                            tp = tpp.tile([128, NKC, 128], bf16, tag="tp",
                                          name="tp")
                            for kc in range(NKC):
                                nc.tensor.transpose(
                                    tp[:, kc, :],
                                    hn[:, kc * 128:(kc + 1) * 128], ident[:])
                            hT = hTp.tile([128, NKC, 128], bf16, tag="hT")
                            nc.vector.tensor_copy(hT[:], tp[:])
                            if not last:
                                mm_retire(hT, 1 + i, g, wr)
                                emit_ags(g, wr)
                            else:
                                mt = accp.tile([128, 258], f32, tag="acc",
                                               name="mt6")
                                for kc in range(NKC):
                                    nc.tensor.matmul(
                                        mt[:, 0:HID], hT[:, kc, :],
                                        Mb_sb[:, 1 + i, kc, 0:HID],
                                        start=(kc == 0), stop=(kc == NKC - 1))
                                h6x = hex_.tile([128, HID], f32, tag="h6x",
                                                name="h6x")
                                nc.vector.tensor_tensor(
                                    out=h6x[:], in0=mt[:, 0:HID],
                                    in1=brow_sb[:, 1 + i, 0:HID], op=ALU.add)
                                rh = hnp.tile([128, HID], bf16, tag="rh",
                                              name="rh")
                                nc.scalar.activation(rh[:], h6x[:], ACT.Relu)
                                tp2 = tpp.tile([128, NKC, 128], bf16,
                                               tag="tp", name="tp2")
                                for kc in range(NKC):
                                    nc.tensor.transpose(
                                        tp2[:, kc, :],
                                        rh[:, kc * 128:(kc + 1) * 128],
                                        ident[:])
                                rhT = hTp.tile([128, NKC, 128], bf16,
                                               tag="hT", name="rhT")
                                nc.vector.tensor_copy(rhT[:], tp2[:])
                                ops = accp.tile([128, 258], f32, tag="acc",
                                                name="ops")
                                for kc in range(NKC):
                                    nc.tensor.matmul(
                                        ops[:, 0:OUT_DIM], rhT[:, kc, :],
                                        W3p_sb[:, kc, :],
                                        start=(kc == 0), stop=(kc == NKC - 1))
                                outf = hex_.tile([128, OUT_DIM], f32,
                                                 tag="outf", name="outf")
                                nc.vector.tensor_copy(outf[:],
                                                      ops[:, 0:OUT_DIM])
                                nc.sync.dma_start(
                                    outD[g * 128:(g + 1) * 128, :], outf[:])
                    if not last:
                        sdst_transpose(wr)

    nc.compile()
    return nc


_CACHE = {}


def kernel(**inputs) -> np.ndarray:
    from concourse.bass_utils import run_bass_kernel_spmd

    cfg = make_cfg()
    ei = np.asarray(inputs["edge_index"])
    pl = plan(cfg, ei)
    key = ("nc", pl["NBT"])
    if key not in _CACHE:
        _CACHE[key] = build(cfg, pl)
    nc = _CACHE[key]
    fw = fold_weights(inputs)
    maps = prep_inputs(cfg, pl, fw, inputs)
    res = run_bass_kernel_spmd(nc, maps, core_ids=list(range(NC)))
    DC = cfg["D_CORE"]
    return np.concatenate([res.results[c]["out"][:DC] for c in range(NC)],
                          0).astype(np.float32)



# revision 33
# speedup vs baseline: 1.0668x; 1.0156x over previous
"""GAT GNN (edge features) Trainium2 kernel — 8-core SPMD, v2.

Sharding: nodes by dst range (6250/core, padded 6400). Table rows are 512B
(256 bf16): per-layer features are kept in a rotated basis h@R_l whose last
column equals Wc_l@att_src_l, so the per-edge s_src logit is just column 255
of the gathered row (no extra embedded scalar -> 512B rows, 33% less gather
and AllGather traffic than 768B). All weight algebra (R_l^{-1} Wc_l R_{l+1}
folds, s_dst columns, We@att_edge projections) is folded on the host; the
per-edge attention bias A = edge_attr @ (We_l att_edge_l) is computed on the
host as well (it is layer-input independent).

Per layer: gather 512B rows (num_idxs_reg-trimmed) -> logits (A + G[:,255]
+ onehot-expanded s_dst via DVE mult+reduce) -> exp/leaky-relu -> Se ->
PSUM numerator+denominator (denominator as column 256 of the same PSUM
tile) -> normalize -> PE-transpose -> fused mm (R^-1 Wc R fold + s_dst
column) -> AllGather into parity ping-pong tables (overlaps next groups).
"""
import sys

sys.path.insert(0, "/opt/trn_rl_repo")

import numpy as np

NEG_SLOPE = 0.2
EPS = 1e-16
NC = 8
HID = 256
EDGE_DIM = 768
OUT_DIM = 256
N_LAYERS = 6


def make_cfg(n_nodes=50000, n_edges=400000):
    c = {}
    c["N"] = n_nodes
    c["E"] = n_edges
    c["D_CORE"] = n_nodes // NC
    c["D_PAD"] = -(-c["D_CORE"] // 128) * 128
    if (c["D_PAD"] // 128) % 2:
        c["D_PAD"] += 128          # even group count so HALF is 128-aligned
    c["HALF"] = c["D_PAD"] // 2
    c["TBL"] = NC * c["HALF"]
    assert c["TBL"] < 32768
    c["NG"] = c["D_PAD"] // 128
    # Each table half is AllGathered in two contiguous pieces so the
    # collective pipeline starts earlier and only the small tail piece
    # (groups G2..NG-1) is exposed at the layer boundary.  Row layout of
    # each half is piece-major (piece A's 8 cores, then piece B's).
    c["G1"] = 13                                      # T0 piece-A groups
    c["AR0"] = c["G1"] * 128
    c["BR0"] = c["HALF"] - c["AR0"]
    # T1 in three pieces (10/8/7 groups) so the collective queue drains
    # and only the last ~7-group piece is exposed at the layer boundary.
    c["G2a"] = (c["NG"] // 2) + 10
    c["G2b"] = (c["NG"] // 2) + 18
    c["AR1"] = 10 * 128                               # T1 piece-A rows/core
    c["AR2"] = 8 * 128                                # T1 piece-B rows/core
    c["BR"] = c["D_PAD"] - c["HALF"] - c["AR1"] - c["AR2"]   # piece-C
    return c


# ---------------- host planner ----------------
def plan(cfg, edge_index, merge=1):
    """Slot space is t-major: slot = ((t*NG + g)*NBT + b)*128 + p.  Gather
    calls cover `merge` consecutive groups of one table half; only the last
    group's trailing pad is trimmed (middle pads gather row 0 harmlessly)."""
    src = np.asarray(edge_index[0], np.int64)
    dst = np.asarray(edge_index[1], np.int64)
    DC, HALF, NG = cfg["D_CORE"], cfg["HALF"], cfg["NG"]
    AR1, AR2, BR = cfg["AR1"], cfg["AR2"], cfg["BR"]
    AR0, BR0 = cfg["AR0"], cfg["BR0"]
    assert NG % merge == 0

    per_core = []
    nbt = 1
    for c in range(NC):
        m = (dst >= c * DC) & (dst < (c + 1) * DC)
        eid = np.nonzero(m)[0]
        es, ed = src[eid], dst[eid] - c * DC
        et = ((es % DC) >= HALF).astype(np.int64)
        cs, ls = es // DC, es % DC
        # Both halves are piece-major (piece A's 8 cores, then piece B's),
        # core-major within each piece (matches the 2-piece AllGathers).
        erow_t0 = np.where(ls < AR0,
                           cs * AR0 + ls,
                           NC * AR0 + cs * BR0 + (ls - AR0))
        erow_t1 = np.where(
            ls < HALF + AR1,
            cs * AR1 + (ls - HALF),
            np.where(ls < HALF + AR1 + AR2,
                     NC * AR1 + cs * AR2 + (ls - HALF - AR1),
                     NC * (AR1 + AR2) + cs * BR + (ls - HALF - AR1 - AR2)))
        erow = np.where(et == 0, erow_t0, erow_t1)
        g = ed // 128
        per_core.append((eid, es, ed, et, erow, g))
        cnt = np.zeros((NG, 2), np.int64)
        np.add.at(cnt, (g, et), 1)
        nbt = max(nbt, int(-(-cnt.max() // 128)), 1)
    NBT = nbt
    NBINS = NG * 2 * NBT
    NSLOT = NBINS * 128
    NJ = NG // merge
    NCALLS = 2 * NJ

    gidx = np.full((NC, NSLOT), -1, np.int16)
    ngrp = np.zeros((NC, 2, NG), np.int32)        # real slots per (t, g)
    onehT = np.zeros((NC, 128, NSLOT), np.int8)   # [dst_local, slot]
    perm = np.full((NC, NSLOT), -1, np.int64)
    for c in range(NC):
        eid, es, ed, et, erow, g = per_core[c]
        for gg in range(NG):
            for t in (0, 1):
                sel = np.nonzero((g == gg) & (et == t))[0]
                base = ((t * NG + gg) * NBT) * 128
                ngrp[c, t, gg] = max(len(sel), 1)
                if len(sel) == 0:
                    continue
                slots = base + np.arange(len(sel))
                gidx[c, slots] = erow[sel].astype(np.int16)
                perm[c, slots] = eid[sel]
                onehT[c, ed[sel] - gg * 128, slots] = 1
    # merged-call trim counts: full middle groups + last group's real count.
    # Ucode contract: num_idxs_reg == count of idx >= 0, and only TRAILING
    # pads may be -1 -> pads below the trim point become row 0 (harmless).
    nreal = np.zeros((NC, NCALLS), np.int32)
    CLM = merge * NBT * 128
    for c in range(NC):
        for t in (0, 1):
            for j in range(NJ):
                nr = ((merge - 1) * NBT * 128
                      + ngrp[c, t, j * merge + merge - 1])
                nreal[c, t * NJ + j] = nr
                s = (t * NG + j * merge) * NBT * 128
                seg = gidx[c, s:s + nr]
                seg[seg < 0] = 0
    return dict(NBT=NBT, NBINS=NBINS, NSLOT=NSLOT, NCALLS=NCALLS,
                MERGE=merge, NJ=NJ, minr=nreal.min(axis=0),
                gidx=gidx, nreal=nreal, oneh=onehT, perm=perm)


def wrap_idx16(gidx, call_len):
    """[NSLOT] -> [128, NSLOT//16] with per-call 16-partition wrap."""
    ncalls = gidx.shape[0] // call_len
    blk = gidx.reshape(ncalls, call_len // 16, 16).transpose(2, 0, 1)
    flat = blk.reshape(16, ncalls * (call_len // 16))
    return np.tile(flat, (8, 1))


# ---------------- host weight folding ----------------
def fold_weights(inputs):
    """R_l rotations + fused per-layer rhs matrices, all in f64.

    Table basis: T^(i) = h^(i) @ R_i, with R_i[:, 255] = Wc_i @ att_src_i so
    s_src == gathered column 255.  R_i = H_i @ diag(1,..,1, beta*n) with H_i
    a Householder reflector, so R_i^{-1} is exact.
    """
    W1 = np.float64(inputs["W1"])
    W2 = np.float64(inputs["W2"])
    Wc = np.float64(inputs["Wc"])
    We = np.float64(inputs["We"])
    a_s = np.float64(inputs["att_src"])
    a_d = np.float64(inputs["att_dst"])
    a_e = np.float64(inputs["att_edge"])
    bias = np.float64(inputs["bias"])
    W3 = np.float64(inputs["W3"])

    R = []
    Rinv = []
    for i in range(N_LAYERS):
        v = Wc[i] @ a_s[i]
        n = np.linalg.norm(v)
        u = v / n
        beta = -1.0 if u[HID - 1] > 0 else 1.0
        w = u.copy()
        w[HID - 1] -= beta
        H = np.eye(HID) - 2.0 * np.outer(w, w) / (w @ w)
        # H @ e_last = beta*u  ->  R[:,255] = H[:,255] * (beta*n) = u*n = v
        Ri = H.copy()
        Ri[:, HID - 1] *= beta * n
        Rii = H.copy()                      # R^-1 = diag(1,..,1/(beta n)) @ H
        Rii[HID - 1, :] /= beta * n
        R.append(Ri)
        Rinv.append(Rii)

    v_d = [Wc[i] @ a_d[i] for i in range(N_LAYERS)]

    # mm matrices: index 0 = h0 producer (x @ W1W2 -> T^(0));
    # index 1+i = applied after layer i's aggregation.
    M = np.zeros((N_LAYERS + 1, HID, 258))
    brow = np.zeros((N_LAYERS + 1, 258))
    W12 = W1 @ W2
    M[0, :, 0:HID] = W12 @ R[0]
    M[0, :, HID] = W12 @ v_d[0]
    for i in range(N_LAYERS - 1):
        M[1 + i, :, 0:HID] = Rinv[i] @ Wc[i] @ R[i + 1]
        M[1 + i, :, HID] = Rinv[i] @ Wc[i] @ v_d[i + 1]
        brow[1 + i, 0:HID] = bias[i] @ R[i + 1]
        brow[1 + i, HID] = bias[i] @ v_d[i + 1]
    M[N_LAYERS, :, 0:HID] = Rinv[N_LAYERS - 1] @ Wc[N_LAYERS - 1]
    brow[N_LAYERS, 0:HID] = bias[N_LAYERS - 1]

    wal = np.einsum("lkh,lh->lk", We, a_e)          # [L, EDGE_DIM]
    W3p = W3[:HID] + W3[HID:]                        # [HID, OUT]
    return dict(R=R, Rinv=Rinv, M=M, brow=brow, wal=wal, W3p=W3p)


# ---------------- host-side input prep ----------------
def prep_inputs(cfg, pl, fw, inputs):
    x = np.asarray(inputs["x"], np.float32)
    ea = np.asarray(inputs["edge_attr"], np.float32)
    DC, DP = cfg["D_CORE"], cfg["D_PAD"]
    NSLOT, NBT, NBINS = pl["NSLOT"], pl["NBT"], pl["NBINS"]
    ml = __import__("ml_dtypes")
    bf16 = ml.bfloat16
    f8 = ml.float8_e4m3

    # per-edge attention bias, all layers at once: [E, L]
    A_full = ea @ np.float32(fw["wal"]).T

    M = np.float32(fw["M"])                          # [7, 256, 258]
    Mb = np.ascontiguousarray(
        M.reshape(N_LAYERS + 1, 2, 128, 258).transpose(2, 0, 1, 3)
    ).astype(bf16)                                   # [128, 7, 2, 258]
    brow = np.ascontiguousarray(
        np.broadcast_to(np.float32(fw["brow"])[None], (128, N_LAYERS + 1, 258))
    ).astype(bf16)
    W3p = np.ascontiguousarray(
        np.float32(fw["W3p"]).reshape(2, 128, OUT_DIM).transpose(1, 0, 2)
    ).astype(bf16)                                   # [128, 2, 256]

    common = dict(Mb=Mb, brow=brow, W3p=W3p)
    maps = []
    for c in range(NC):
        xs = np.zeros((DP, HID), np.float32)
        xs[:DC] = x[c * DC:(c + 1) * DC]
        m = dict(common)
        m["xT"] = np.ascontiguousarray(xs.T).astype(bf16)
        m["gidx"] = wrap_idx16(pl["gidx"][c], pl["MERGE"] * NBT * 128)
        m["nreal"] = pl["nreal"][c][None, :].astype(np.int32)
        # A in device layout [128, L, NSLOT//128]
        Ac = np.zeros((NSLOT, N_LAYERS), np.float32)
        real = pl["perm"][c] >= 0
        Ac[real] = A_full[pl["perm"][c][real]]
        m["A"] = np.ascontiguousarray(
            Ac.reshape(NSLOT // 128, 128, N_LAYERS).transpose(1, 2, 0)
        ).astype(bf16)
        # per-bin transposed onehot [slot_in_bin(p), dst_col], f8
        oh = pl["oneh"][c]
        oh_se = np.zeros((128, NSLOT), np.int8)
        for b in range(NBINS):
            oh_se[:, b * 128:(b + 1) * 128] = oh[:, b * 128:(b + 1) * 128].T
        m["oneh"] = oh_se.astype(f8)
        maps.append(m)
    return maps


# ---------------- numpy emulation (plan/fold validation) ----------------
def emulate(cfg, inputs, pl, fw):
    x = np.asarray(inputs["x"], np.float32)
    ea = np.asarray(inputs["edge_attr"], np.float32)
    DC, DP, HALF, TBL, NG = (cfg["D_CORE"], cfg["D_PAD"], cfg["HALF"],
                             cfg["TBL"], cfg["NG"])
    NSLOT, NBT = pl["NSLOT"], pl["NBT"]
    M = np.float32(fw["M"])
    brow = np.float32(fw["brow"])
    W3p = np.float32(fw["W3p"])

    A_full = ea @ np.float32(fw["wal"]).T
    A = np.zeros((NC, NSLOT, N_LAYERS), np.float32)
    for c in range(NC):
        real = pl["perm"][c] >= 0
        A[c][real] = A_full[pl["perm"][c][real]]

    # h0 phase
    mt = np.zeros((NC, DP, 257), np.float32)
    for c in range(NC):
        xs = np.zeros((DP, HID), np.float32)
        xs[:DC] = x[c * DC:(c + 1) * DC]
        mt[c] = xs @ M[0, :, 0:257] + brow[0, 0:257]

    slot_g = (np.arange(NSLOT) // (128 * NBT)) % NG
    out = np.zeros((NC, DP, OUT_DIM), np.float32)
    for i in range(N_LAYERS):
        # tables from mt
        agin = mt[:, :, 0:HID]
        sdst = mt[:, :, HID]
        AR1, AR2, BR = cfg["AR1"], cfg["AR2"], cfg["BR"]
        AR0, BR0 = cfg["AR0"], cfg["BR0"]
        T0 = np.concatenate(
            [agin[:, :AR0].reshape(NC * AR0, HID),
             agin[:, AR0:HALF].reshape(NC * BR0, HID)], 0)
        T1 = np.concatenate(
            [agin[:, HALF:HALF + AR1].reshape(NC * AR1, HID),
             agin[:, HALF + AR1:HALF + AR1 + AR2].reshape(NC * AR2, HID),
             agin[:, HALF + AR1 + AR2:].reshape(NC * BR, HID)], 0)
        mt2 = np.zeros((NC, DP, 257), np.float32)
        for c in range(NC):
            gi = pl["gidx"][c].astype(np.int64)
            valid = pl["perm"][c] >= 0
            slot_t = np.arange(NSLOT) // (NG * NBT * 128)
            G = np.zeros((NSLOT, HID), np.float32)
            G[valid & (slot_t == 0)] = T0[gi[valid & (slot_t == 0)]]
            G[valid & (slot_t == 1)] = T1[gi[valid & (slot_t == 1)]]
            ssrc = G[:, HID - 1]
            oh = pl["oneh"][c].astype(np.float32)    # [dst_local, slot]
            sdsel = np.zeros(NSLOT, np.float32)
            for gg in range(NG):
                sl = slot_g == gg
                sdsel[sl] = oh[:, sl].T @ sdst[c, gg * 128:(gg + 1) * 128]
            alpha = ssrc + sdsel + A[c, :, i]
            eac = np.maximum(np.exp(alpha), np.exp(NEG_SLOPE * alpha))
            U = np.zeros((DP, HID), np.float32)
            dns = np.zeros(DP, np.float32)
            Se = oh * eac[None, :]
            for gg in range(NG):
                sl = slot_g == gg
                U[gg * 128:(gg + 1) * 128] = Se[:, sl] @ G[sl]
                dns[gg * 128:(gg + 1) * 128] = Se[:, sl].sum(1)
            U = U / (dns + EPS)[:, None]
            if i < N_LAYERS - 1:
                mt2[c] = U @ M[1 + i, :, 0:257] + brow[1 + i, 0:257]
                mt2[c, DC:] = 0.0
            else:
                h7 = U @ M[1 + i, :, 0:HID] + brow[1 + i, 0:HID]
                out[c] = np.maximum(h7, 0.0) @ W3p
        mt = mt2
    return np.concatenate([out[c, :DC] for c in range(NC)], 0)


# ---------------- device kernel ----------------
def build(cfg, pl, queues=4, debug_taps=False, zero_g=False, no_ag=False,
          gq=4):
    import concourse.bass as bass
    import concourse.tile as tile
    import concourse.mybir as mybir
    from concourse import bacc
    from concourse.masks import make_identity

    f32, bf16, i16, i32 = (mybir.dt.float32, mybir.dt.bfloat16,
                           mybir.dt.int16, mybir.dt.int32)
    f8 = mybir.dt.float8e4
    ACT = mybir.ActivationFunctionType
    ALU = mybir.AluOpType

    DP, HALF, TBL, NG = cfg["D_PAD"], cfg["HALF"], cfg["TBL"], cfg["NG"]
    G2a, G2b = cfg["G2a"], cfg["G2b"]
    AR1, AR2 = cfg["AR1"], cfg["AR2"]
    G1, AR0 = cfg["G1"], cfg["AR0"]
    NBT, NSLOT, NCALLS = pl["NBT"], pl["NSLOT"], pl["NCALLS"]
    CL = NBT * 128
    NKC = HID // 128
    NJ = NSLOT // 128

    nc = bacc.Bacc(None, target_bir_lowering=False, debug=False,
                   num_swdge_queues=queues)

    # inputs
    xT = nc.dram_tensor("xT", [HID, DP], bf16, kind="ExternalInput")
    gidxD = nc.dram_tensor("gidx", [128, NSLOT // 16], i16, kind="ExternalInput")
    nrealD = nc.dram_tensor("nreal", [1, NCALLS], i32, kind="ExternalInput")
    onehD = nc.dram_tensor("oneh", [128, NSLOT], f8, kind="ExternalInput")
    AD = nc.dram_tensor("A", [128, N_LAYERS, NJ], bf16, kind="ExternalInput")
    MbD = nc.dram_tensor("Mb", [128, N_LAYERS + 1, NKC, 258], bf16,
                         kind="ExternalInput")
    browD = nc.dram_tensor("brow", [128, N_LAYERS + 1, 258], bf16,
                           kind="ExternalInput")
    W3pD = nc.dram_tensor("W3p", [128, NKC, OUT_DIM], bf16,
                          kind="ExternalInput")
    outD = nc.dram_tensor("out", [DP, OUT_DIM], f32, kind="ExternalOutput")
    dbg = {}
    if debug_taps:
        for nm, shp, dt in [("dbg_T0", [TBL, HID], bf16),
                            ("dbg_srep", [128, 128], bf16),
                            ("dbg_sc", [128, 16 * NBT], f32),
                            ("dbg_G", [128, NBT * HID], bf16),
                            ("dbg_Se", [128, 2 * NBT * 128], bf16),
                            ("dbg_gps", [128, 258], f32),
                            ("dbg_hn", [128, HID], bf16),
                            ("dbg_mt", [128, 258], f32)]:
            dbg[nm] = nc.dram_tensor(nm, shp, dt, kind="ExternalOutput")

    # internals (ping-pong tables/agin by layer parity)
    aginD = [nc.dram_tensor(f"agin{p}", [DP, HID], bf16) for p in (0, 1)]
    T0D = [nc.dram_tensor(f"T0_{p}", [TBL, HID], bf16, addr_space="Shared")
           for p in (0, 1)]
    T1D = [nc.dram_tensor(f"T1_{p}", [TBL, HID], bf16, addr_space="Shared")
           for p in (0, 1)]
    sdTD = [nc.dram_tensor(f"sdT{p}", [NG, 128], bf16) for p in (0, 1)]

    rg = [list(range(NC))]

    with tile.TileContext(nc) as tc:
        with (
            tc.tile_pool(name="res", bufs=1) as res,
            tc.tile_pool(name="lw", bufs=4) as lw,
            tc.tile_pool(name="gp", bufs=10) as gp,
            tc.tile_pool(name="sep", bufs=6) as sep,
            tc.tile_pool(name="exm", bufs=4) as exmp,
            tc.tile_pool(name="sc", bufs=6) as scp,
            tc.tile_pool(name="hn", bufs=5) as hnp,
            tc.tile_pool(name="hT", bufs=5) as hTp,
            tc.tile_pool(name="hex", bufs=5) as hex_,
            tc.tile_pool(name="acc", bufs=4, space="PSUM") as accp,
            tc.tile_pool(name="dns", bufs=2, space="PSUM") as dnsp,
            tc.tile_pool(name="tpp", bufs=2, space="PSUM") as tpp,
        ):
            # resident inputs
            gidx_sb = res.tile([128, NSLOT // 16], i16)
            nc.sync.dma_start(gidx_sb[:], gidxD[:])
            nreal_sb = res.tile([1, NCALLS], i32)
            nc.sync.dma_start(nreal_sb[:], nrealD[:])
            oneh_sb = res.tile([128, NSLOT], f8)
            nc.sync.dma_start(oneh_sb[:], onehD[:])
            A_sb = res.tile([128, N_LAYERS, NJ], bf16)
            nc.sync.dma_start(A_sb[:], AD[:])
            Mb_sb = res.tile([128, N_LAYERS + 1, NKC, 258], bf16)
            nc.sync.dma_start(Mb_sb[:], MbD[:])
            brow_sb = res.tile([128, N_LAYERS + 1, 258], bf16)
            nc.sync.dma_start(brow_sb[:], browD[:])
            W3p_sb = res.tile([128, NKC, OUT_DIM], bf16)
            nc.sync.dma_start(W3p_sb[:], W3pD[:])

            ident = res.tile([128, 128], bf16)
            make_identity(nc, ident[:])
            ones_col = res.tile([128, 1], bf16)
            nc.vector.memset(ones_col[:], 1.0)
            sdst_bf = res.tile([128, NG], bf16)
            nreal_reg = nc.gpsimd.alloc_register("nreal_reg")

            def zero_pad_suffix(G, call, nbins):
                """Sim-only: zero pad slots (logical tiles are NaN there).
                On HW the pool priming below keeps stale pads finite, which
                is all the masked (oneh=0) reads need."""
                if not zero_g:
                    return
                b0 = int(pl["minr"][call]) // 128
                if b0 < nbins:
                    nc.vector.memset(
                        G[:, b0:nbins, :].rearrange("p a b -> p (a b)"), 0.0)
            if not zero_g:
                for _ in range(10):
                    gt = gp.tile([128, pl["MERGE"] * NBT, HID], bf16, tag="G",
                                 name="gprime")
                    nc.vector.memset(gt[:].rearrange("p a b -> p (a b)"), 0.0)

            def mm_retire(src_sb, li, g, wr_parity):
                """matmul src^T @ M[li] (+brow) -> table row + sdst col."""
                mt = accp.tile([128, 258], f32, tag="acc", name="mt")
                for kc in range(NKC):
                    nc.tensor.matmul(mt[:, 0:257], src_sb[:, kc, :],
                                     Mb_sb[:, li, kc, 0:257],
                                     start=(kc == 0), stop=(kc == NKC - 1))
                hx = hex_.tile([128, HID], bf16, tag="hx")
                nc.vector.tensor_tensor(out=hx[:], in0=mt[:, 0:HID],
                                        in1=brow_sb[:, li, 0:HID], op=ALU.add)
                nc.vector.tensor_tensor(out=sdst_bf[:, g:g + 1],
                                        in0=mt[:, HID:HID + 1],
                                        in1=brow_sb[:, li, HID:HID + 1],
                                        op=ALU.add)
                nc.sync.dma_start(aginD[wr_parity][g * 128:(g + 1) * 128, :],
                                  hx[:])

            def emit_ags(g, wr_parity):
                if no_ag:
                    return
                if g == G1 - 1:
                    nc.gpsimd.collective_compute(
                        "AllGather", ALU.bypass, replica_groups=rg,
                        ins=[aginD[wr_parity][0:AR0, :]],
                        outs=[T0D[wr_parity][0:NC * AR0, :]])
                if g == NG // 2 - 1:
                    nc.gpsimd.collective_compute(
                        "AllGather", ALU.bypass, replica_groups=rg,
                        ins=[aginD[wr_parity][AR0:HALF, :]],
                        outs=[T0D[wr_parity][NC * AR0:TBL, :]])
                if g == G2a - 1:
                    nc.gpsimd.collective_compute(
                        "AllGather", ALU.bypass, replica_groups=rg,
                        ins=[aginD[wr_parity][HALF:HALF + AR1, :]],
                        outs=[T1D[wr_parity][0:NC * AR1, :]])
                if g == G2b - 1:
                    nc.gpsimd.collective_compute(
                        "AllGather", ALU.bypass, replica_groups=rg,
                        ins=[aginD[wr_parity][HALF + AR1:HALF + AR1 + AR2, :]],
                        outs=[T1D[wr_parity][NC * AR1:NC * (AR1 + AR2), :]])
                if g == NG - 1:
                    nc.gpsimd.collective_compute(
                        "AllGather", ALU.bypass, replica_groups=rg,
                        ins=[aginD[wr_parity][HALF + AR1 + AR2:DP, :]],
                        outs=[T1D[wr_parity][NC * (AR1 + AR2):TBL, :]])

            def sdst_transpose(parity):
                sdT_ps = tpp.tile([128, 128], bf16, tag="tp", name="sdT_ps")
                nc.tensor.transpose(sdT_ps[0:NG, :], sdst_bf[:], ident[:])
                sdT = hTp.tile([128, 128], bf16, tag="sdT", name="sdT")
                nc.vector.tensor_copy(sdT[0:NG, :], sdT_ps[0:NG, :])
                nc.sync.dma_start(sdTD[parity][:], sdT[0:NG, :])

            # ---------- h0: T^(0) = x @ W1W2R0 ----------
            with nc.named_scope("h0"):
                for g in range(NG):
                    xt = lw.tile([128, NKC, 128], bf16, tag="xt")
                    for kc in range(NKC):
                        nc.sync.dma_start(
                            xt[:, kc, :],
                            xT[kc * 128:(kc + 1) * 128, g * 128:(g + 1) * 128])
                    mm_retire(xt, 0, g, 0)
                    emit_ags(g, 0)
                sdst_transpose(0)

            # ---------- layers ----------
            MERGE, NJ = pl["MERGE"], pl["NJ"]
            CLM = MERGE * CL
            for i in range(N_LAYERS):
                last = i == N_LAYERS - 1
                rd, wr = i % 2, (i + 1) % 2
                if debug_taps and i == 0:
                    nc.sync.dma_start(dbg["dbg_T0"][:], T0D[0][:])
                with nc.named_scope(f"eg{i}"):
                    for j in range(NJ):
                        Gm = [None, None]
                        for t in (0, 1):
                            call = t * NJ + j
                            G = gp.tile([128, MERGE * NBT, HID], bf16,
                                        tag="G")
                            zero_pad_suffix(G, call, MERGE * NBT)
                            nc.gpsimd.reg_load(nreal_reg,
                                               nreal_sb[0:1, call:call + 1])
                            nc.gpsimd.dma_gather(
                                out_ap=G[:],
                                in_ap=(T0D[rd][:] if t == 0 else T1D[rd][:]),
                                idxs_ap=gidx_sb[:, call * (CLM // 16):
                                                (call + 1) * (CLM // 16)],
                                num_idxs=CLM, num_idxs_reg=nreal_reg,
                                elem_size=HID,
                                queue_num=(t * 2 + (j % 2)) % gq)
                            Gm[t] = G
                        for gsub in range(MERGE):
                            g = j * MERGE + gsub
                            dbg_this = debug_taps and i == 0 and g == 0
                            srep = lw.tile([128, 128], bf16, tag="srep",
                                           name="srep")
                            nc.sync.dma_start(
                                srep[:],
                                sdTD[rd][g:g + 1, :].to_broadcast((128, 128)))
                            if dbg_this:
                                nc.sync.dma_start(dbg["dbg_srep"][:], srep[:])
                            gps = accp.tile([128, 258], f32, tag="acc",
                                            name="gps")
                            dns = dnsp.tile([128, 1], f32, tag="dns",
                                            name="dns")
                            alpha2 = scp.tile([128, 2 * NBT], f32,
                                              tag="alpha2")
                            oh_vs = [None, None]
                            Gvs = [None, None]
                            for t in (0, 1):
                                bb = (t * NG + g) * NBT
                                Gv = Gm[t][:, gsub * NBT:(gsub + 1) * NBT, :]
                                oh_v = oneh_sb[:, bb * 128:(bb + NBT) * 128] \
                                    .rearrange("p (a b) -> p a b", b=128)
                                oh_vs[t] = oh_v
                                Gvs[t] = Gv
                                exm = exmp.tile([128, NBT, 128], bf16,
                                                tag="exm")
                                nc.vector.tensor_tensor(
                                    out=exm[:], in0=oh_v,
                                    in1=srep[:, None, :]
                                    .to_broadcast([128, NBT, 128]),
                                    op=ALU.mult)
                                ex = scp.tile([128, NBT], f32, tag=f"ex{t}")
                                nc.vector.tensor_reduce(
                                    ex[:], exm[:], axis=mybir.AxisListType.X,
                                    op=ALU.add)
                                beta = scp.tile([128, NBT], f32,
                                                tag=f"beta{t}")
                                nc.vector.tensor_tensor(
                                    out=beta[:],
                                    in0=A_sb[:, i, bb:bb + NBT],
                                    in1=Gv[:, :, HID - 1], op=ALU.add)
                                nc.vector.tensor_tensor(
                                    out=alpha2[:, t * NBT:(t + 1) * NBT],
                                    in0=beta[:], in1=ex[:], op=ALU.add)
                            e1 = scp.tile([128, 2 * NBT], f32, tag="e1")
                            nc.scalar.activation(e1[:], alpha2[:], ACT.Exp)
                            e2 = scp.tile([128, 2 * NBT], f32, tag="e2")
                            nc.scalar.activation(e2[:], alpha2[:], ACT.Exp,
                                                 scale=NEG_SLOPE)
                            eac = scp.tile([128, 2 * NBT], f32, tag="eac")
                            nc.vector.tensor_tensor(
                                out=eac[:], in0=e1[:], in1=e2[:], op=ALU.max)
                            for t in (0, 1):
                                oh_v = oh_vs[t]
                                Gv = Gvs[t]
                                Se = sep.tile([128, NBT, 128], bf16,
                                              tag=f"Se{t}")
                                for b in range(NBT):
                                    nc.scalar.activation(
                                        Se[:, b, :], oh_v[:, b, :],
                                        ACT.Copy,
                                        scale=eac[:, t * NBT + b:
                                                  t * NBT + b + 1])
                                if dbg_this:
                                    sc = dbg["dbg_sc"]
                                    nc.sync.dma_start(
                                        sc[:, 4 * NBT:6 * NBT], alpha2[:])
                                    nc.sync.dma_start(
                                        sc[:, 6 * NBT:8 * NBT], eac[:])
                                    nc.sync.dma_start(
                                        dbg["dbg_Se"][:, t * NBT * 128:
                                                      (t + 1) * NBT * 128],
                                        Se[:].rearrange("p a b -> p (a b)"))
                                    if t == 0:
                                        nc.sync.dma_start(
                                            dbg["dbg_G"][:],
                                            Gv[:].rearrange(
                                                "p a b -> p (a b)"))
                                for b in range(NBT):
                                    ii = t * NBT + b
                                    nc.tensor.matmul(
                                        gps[:, 0:HID], Se[:, b, :],
                                        Gv[:, b, :],
                                        start=(ii == 0),
                                        stop=(ii == 2 * NBT - 1))
                                    nc.tensor.matmul(
                                        dns[:], Se[:, b, :], ones_col[:],
                                        start=(ii == 0),
                                        stop=(ii == 2 * NBT - 1))
                            dcol = scp.tile([128, 1], f32, tag="dcol")
                            nc.vector.tensor_scalar_add(dcol[:], dns[:], EPS)
                            rcol = scp.tile([128, 1], f32, tag="rcol")
                            nc.vector.reciprocal(rcol[:], dcol[:])
                            hn = hnp.tile([128, HID], bf16, tag="hn")
                            if dbg_this:
                                gcp = hex_.tile([128, 258], f32, tag="gcp",
                                                name="gcp")
                                nc.vector.tensor_copy(gcp[:, 0:HID],
                                                      gps[:, 0:HID])
                                nc.vector.tensor_copy(gcp[:, HID:HID + 1],
                                                      dns[:])
                                nc.sync.dma_start(dbg["dbg_gps"][:], gcp[:])
                            nc.scalar.activation(hn[:], gps[:, 0:HID],
                                                 ACT.Copy, scale=rcol[:, 0:1])
                            if dbg_this:
                                nc.sync.dma_start(dbg["dbg_hn"][:], hn[:])
                            tp = tpp.tile([128, NKC, 128], bf16, tag="tp",
                                          name="tp")
                            for kc in range(NKC):
                                nc.tensor.transpose(
                                    tp[:, kc, :],
                                    hn[:, kc * 128:(kc + 1) * 128], ident[:])
                            hT = hTp.tile([128, NKC, 128], bf16, tag="hT")
                            nc.vector.tensor_copy(hT[:], tp[:])
                            if not last:
                                mm_retire(hT, 1 + i, g, wr)
                                emit_ags(g, wr)
                            else:
                                mt = accp.tile([128, 258], f32, tag="acc",
                                               name="mt6")
                                for kc in range(NKC):
                                    nc.tensor.matmul(
                                        mt[:, 0:HID], hT[:, kc, :],
                                        Mb_sb[:, 1 + i, kc, 0:HID],
                                        start=(kc == 0), stop=(kc == NKC - 1))
                                h6x = hex_.tile([128, HID], f32, tag="h6x",
                                                name="h6x")
                                nc.vector.tensor_tensor(
                                    out=h6x[:], in0=mt[:, 0:HID],
                                    in1=brow_sb[:, 1 + i, 0:HID], op=ALU.add)
                                rh = hnp.tile([128, HID], bf16, tag="rh",
                                              name="rh")
                                nc.scalar.activation(rh[:], h6x[:], ACT.Relu)
                                tp2 = tpp.tile([128, NKC, 128], bf16,
                                               tag="tp", name="tp2")
                                for kc in range(NKC):
                                    nc.tensor.transpose(
                                        tp2[:, kc, :],
                                        rh[:, kc * 128:(kc + 1) * 128],
                                        ident[:])
                                rhT = hTp.tile([128, NKC, 128], bf16,
                                               tag="hT", name="rhT")
                                nc.vector.tensor_copy(rhT[:], tp2[:])
                                ops = accp.tile([128, 258], f32, tag="acc",
                                                name="ops")
                                for kc in range(NKC):
                                    nc.tensor.matmul(
                                        ops[:, 0:OUT_DIM], rhT[:, kc, :],
                                        W3p_sb[:, kc, :],
                                        start=(kc == 0), stop=(kc == NKC - 1))
                                outf = hex_.tile([128, OUT_DIM], f32,
                                                 tag="outf", name="outf")
                                nc.vector.tensor_copy(outf[:],
                                                      ops[:, 0:OUT_DIM])
                                nc.sync.dma_start(
                                    outD[g * 128:(g + 1) * 128, :], outf[:])
                    if not last:
                        sdst_transpose(wr)

    nc.compile()
    return nc


_CACHE = {}


def kernel(**inputs) -> np.ndarray:
    from concourse.bass_utils import run_bass_kernel_spmd

    cfg = make_cfg()
    ei = np.asarray(inputs["edge_index"])
    pl = plan(cfg, ei)
    key = ("nc", pl["NBT"])
    if key not in _CACHE:
        _CACHE[key] = build(cfg, pl)
    nc = _CACHE[key]
    fw = fold_weights(inputs)
    maps = prep_inputs(cfg, pl, fw, inputs)
    res = run_bass_kernel_spmd(nc, maps, core_ids=list(range(NC)))
    DC = cfg["D_CORE"]
    return np.concatenate([res.results[c]["out"][:DC] for c in range(NC)],
                          0).astype(np.float32)

